# revision 1
# baseline (speedup 1.0000x reference)
"""GCN binding-affinity model on 8 Trainium2 NeuronCores.

Self-contained: builds a Bass/Tile SPMD program, shards the graph across 8
cores (nodes dst-partitioned; per-core edges packed into fixed 128-edge tiles
per 128-node window, split by src half for int16 dma_gather indices), runs via
bass_utils.run_bass_kernel_spmd, and returns the full [256, 1] output.

Math (equivalent to the reference):
  per layer: agg = dinv * ((A + I) @ (dinv * t)) with t the passed features,
  where dinv = rsqrt(indegree + 1).  Self-loops are explicit edges.
  L1 passes t = x (scalar; table stores y = dinv*x replicated to 64 lanes),
  then h1 = relu(dinv*S1*W1 + b1) via K=1 outer product.
  L2 passes t = h1 (64-d), then h2 = relu((S2*dinv) @ W2 + b2).
  L3 passes t = h2 @ W3 (64-d), then h3 = relu(S3*dinv + b3).
  pool: one-hot matmul by graph id + AllReduce + mean + 2-layer MLP.

Gather: per group of GSZ windows, one multi-packet dma_gather per src half
into disjoint chunk ranges of one SBUF tile, then a 128-idx single-packet
fence gather on the same SWDGE queue (per-engine FIFO => fence completion
implies all prior descriptors landed); every consumer matmul gets an explicit
dep on the fence.
"""

import os
import sys
from contextlib import ExitStack

import numpy as np

for _p in ("/opt/trn_rl_repo",):
    if _p not in sys.path and os.path.isdir(_p):
        sys.path.insert(0, _p)

import concourse.bass as bass
import concourse.mybir as mybir
import concourse.tile as tile
from concourse import bacc
from concourse import bass_utils
from concourse.masks import make_identity
from concourse.tile_rust import add_dep_helper

F32 = mybir.dt.float32
I16 = mybir.dt.int16
AF = mybir.ActivationFunctionType
OP = mybir.AluOpType
GSZ = 4  # windows per gather group


class Cfg:
    def __init__(self, n_nodes=50000, n_edges=600000, n_graphs=256,
                 n_cores=8, nw=49, nt_a=11, nt_b=7, half=32768, gsz=GSZ):
        self.N = n_nodes
        self.E = n_edges
        self.G = n_graphs
        self.C = n_cores
        self.NW = nw               # 128-node windows per core
        self.NT_A = nt_a           # tiles/window for src < half
        self.NT_B = nt_b           # tiles/window for src >= half
        self.NT = nt_a + nt_b
        self.HALF = half
        self.NWS = nw * 128
        self.PAD_N = self.C * self.NWS
        assert self.PAD_N >= n_nodes
        assert self.PAD_N - half < 32768 and half <= 32768
        self.GP = 256
        assert n_graphs <= self.GP
        # gather groups: [start_window, n_windows]
        self.GSZ = gsz
        self.groups = [(s, min(gsz, nw - s)) for s in range(0, nw, gsz)]

    def key(self):
        return (self.N, self.E, self.G, self.C, self.NW, self.NT_A,
                self.NT_B, self.HALF, self.GSZ)


def chunk_of(cfg, wl, t, gn):
    """gb chunk index for window-in-group wl, window-tile t, group size gn."""
    if t < cfg.NT_A:
        return wl * cfg.NT_A + t
    return gn * cfg.NT_A + wl * cfg.NT_B + (t - cfg.NT_A)


# ----------------------------------------------------------------------------
# Host-side sharding / packing
# ----------------------------------------------------------------------------

def compute_caps(n, edge_index, cfg_proto):
    """Data-driven NT_A/NT_B (max tiles needed per window + margin)."""
    half = cfg_proto.HALF
    src = np.asarray(edge_index[0], dtype=np.int64)
    dst = np.asarray(edge_index[1], dtype=np.int64)
    self_nodes = np.arange(n, dtype=np.int64)
    a_src = np.concatenate([src, self_nodes])
    a_dst = np.concatenate([dst, self_nodes])
    win = a_dst >> 7  # global 128-node window
    n_win = cfg_proto.C * cfg_proto.NW
    isa = a_src < half
    cnt_a = np.bincount(win[isa], minlength=n_win)
    cnt_b = np.bincount(win[~isa], minlength=n_win)
    nt_a = int(np.ceil(cnt_a.max() / 128))
    nt_b = max(int(np.ceil(cnt_b.max() / 128)), 1)
    return nt_a, nt_b


def wrap16(vals):
    """sequence position i -> [i % 16, i // 16], int16."""
    return np.ascontiguousarray(vals.reshape(-1, 16).T.astype(np.int16))


def rep8(block):
    """replicate a [16, X] int16 block to [128, X] (8 Q7 core groups)."""
    return np.tile(block, (8, 1))


def prep_inputs(cfg, x, W1, b1, W2, b2, W3, b3, lin1_w, lin1_b, lin2_w,
                lin2_b, edge_index, batch):
    N, C, NW, NWS = cfg.N, cfg.C, cfg.NW, cfg.NWS
    NT_A, NT_B, NT, HALF = cfg.NT_A, cfg.NT_B, cfg.NT, cfg.HALF

    src = np.asarray(edge_index[0], dtype=np.int64)
    dst = np.asarray(edge_index[1], dtype=np.int64)
    batch = np.asarray(batch, dtype=np.int64)
    x = np.asarray(x, dtype=np.float32).reshape(-1)

    deg = np.bincount(dst, minlength=N).astype(np.float32)
    x_ext = np.zeros(cfg.PAD_N, np.float32); x_ext[:N] = x
    deg_ext = np.zeros(cfg.PAD_N, np.float32); deg_ext[:N] = deg
    batch_ext = np.full(cfg.PAD_N, -1.0, np.float32)
    batch_ext[:N] = batch.astype(np.float32)

    iota = np.broadcast_to(np.arange(cfg.GP, dtype=np.float32),
                           (128, cfg.GP)).copy()
    cnts = np.bincount(batch, minlength=cfg.GP).astype(np.float32)
    cnts2 = np.ascontiguousarray(cnts.reshape(2, 128).T)  # [128, half]

    # append self-loops, assign to cores by dst
    self_nodes = np.arange(N, dtype=np.int64)
    a_src = np.concatenate([src, self_nodes])
    a_dst = np.concatenate([dst, self_nodes])
    core_of = a_dst // NWS
    order0 = np.argsort(core_of, kind="stable")
    a_src, a_dst, core_sorted = a_src[order0], a_dst[order0], core_of[order0]

    # per-group column offsets in the idx arrays (groups may differ in size)
    colsA_of = [gn * NT_A * 128 // 16 for _, gn in cfg.groups]
    colsB_of = [gn * NT_B * 128 // 16 for _, gn in cfg.groups]
    offA = np.concatenate([[0], np.cumsum(colsA_of)]).astype(int)
    offB = np.concatenate([[0], np.cumsum(colsB_of)]).astype(int)
    TOT_A, TOT_B = int(offA[-1]), int(offB[-1])

    in_maps = []
    for c in range(C):
        base = c * NWS
        lo = np.searchsorted(core_sorted, c, side="left")
        hi = np.searchsorted(core_sorted, c, side="right")
        c_src, c_dst = a_src[lo:hi], a_dst[lo:hi]
        w_of = (c_dst - base) >> 7
        is_a = c_src < HALF
        # order edges by (window, half), then rank within each bucket
        key = w_of * 2 + (~is_a)
        order = np.argsort(key, kind="stable")
        c_src, c_dst, w_of, is_a = (c_src[order], c_dst[order], w_of[order],
                                    is_a[order])
        key = key[order]
        kstart = np.searchsorted(key, np.arange(2 * NW), side="left")
        pos = np.arange(len(key)) - kstart[key]
        cnt = np.searchsorted(key, np.arange(2 * NW), side="right") - kstart
        if cnt[0::2].max(initial=0) > NT_A * 128 or \
           cnt[1::2].max(initial=0) > NT_B * 128:
            raise ValueError("window half overflow; increase caps")

        # per-window slot arrays: slots [0, NT_A*128) half A, rest half B
        slot_idx = np.zeros((NW, NT * 128), np.int64)      # biased table row
        dstrel = np.full((NW, NT * 128), -1.0, np.float32)
        wslot = np.where(is_a, pos, NT_A * 128 + pos)
        slot_idx[w_of, wslot] = np.where(is_a, c_src, c_src - HALF)
        dstrel[w_of, wslot] = (c_dst - base - (w_of << 7)).astype(np.float32)

        ixa = np.zeros((128, TOT_A), np.int16)
        ixb = np.zeros((128, TOT_B), np.int16)
        for gi, (ws, gn) in enumerate(cfg.groups):
            seq_a = slot_idx[ws:ws + gn, :NT_A * 128].reshape(-1)
            seq_b = slot_idx[ws:ws + gn, NT_A * 128:].reshape(-1)
            ixa[:, offA[gi]:offA[gi + 1]] = rep8(wrap16(seq_a))
            ixb[:, offB[gi]:offB[gi + 1]] = rep8(wrap16(seq_b))

        # dstrel as SBUF layout [128, NW*NT]: col w*NT+t, partition p
        drel = np.ascontiguousarray(
            dstrel.reshape(NW * NT, 128).T.astype(np.float32))

        sl = slice(base, base + NWS)
        nd = lambda a: np.ascontiguousarray(a[sl].reshape(NW, 128).T)
        degT = np.broadcast_to(deg_ext[sl][None, :], (128, NWS)).copy()

        in_maps.append({
            "ixa": ixa, "ixb": ixb,
            "ei_dstrel": drel,
            "nd_deg": nd(deg_ext),
            "nd_x": nd(x_ext),
            "nd_batch": nd(batch_ext),
            "degT": degT,
            "iota": iota,
            "cnts": cnts2,
            "w1": np.asarray(W1, np.float32).reshape(1, 64),
            "w2": np.asarray(W2, np.float32).reshape(64, 128),
            "w3": np.asarray(W3, np.float32).reshape(128, 64),
            "b1": np.asarray(b1, np.float32).reshape(64, 1),
            "b2": np.asarray(b2, np.float32).reshape(128, 1),
            "b3": np.asarray(b3, np.float32).reshape(64, 1),
            "l1w": np.asarray(lin1_w, np.float32).reshape(64, 32),
            "l1b": np.asarray(lin1_b, np.float32).reshape(32, 1),
            "l2w": np.asarray(lin2_w, np.float32).reshape(32, 1),
            "l2b": np.full((128, 1),
                           np.float32(np.asarray(lin2_b).reshape(())),
                           np.float32),
        })
    return in_maps, (TOT_A, TOT_B, offA, offB)


# ----------------------------------------------------------------------------
# Device program
# ----------------------------------------------------------------------------

def build_program(cfg, TOT_A, TOT_B, offA, offB, reps=1, no_coll=False):
    NW, NT, NWS, PAD_N, GP = cfg.NW, cfg.NT, cfg.NWS, cfg.PAD_N, cfg.GP
    NT_A, NT_B, HALF = cfg.NT_A, cfg.NT_B, cfg.HALF
    rg = [list(range(cfg.C))]

    nc = bacc.Bacc("TRN2", target_bir_lowering=False, debug=False,
                   num_devices=cfg.C)

    din = {}
    def inp(name, shape, dt=F32):
        din[name] = nc.dram_tensor(name, list(shape), dt, kind="ExternalInput")
        return din[name]

    inp("ixa", (128, TOT_A), I16)
    inp("ixb", (128, TOT_B), I16)
    inp("ei_dstrel", (128, NW * NT))
    inp("nd_deg", (128, NW))
    inp("nd_x", (128, NW))
    inp("nd_batch", (128, NW))
    inp("degT", (128, NWS))
    inp("iota", (128, GP))
    inp("cnts", (128, 2))
    inp("w1", (1, 64)); inp("w2", (64, 128)); inp("w3", (128, 64))
    inp("b1", (64, 1)); inp("b2", (128, 1)); inp("b3", (64, 1))
    inp("l1w", (64, 32)); inp("l1b", (32, 1)); inp("l2w", (32, 1))
    inp("l2b", (128, 1))

    out_d = nc.dram_tensor("out", [cfg.GP, 1], F32, kind="ExternalOutput")

    y_sl = nc.dram_tensor("y_slice", [NWS, 64], F32, kind="Internal")
    y_full = nc.dram_tensor("y_full", [PAD_N, 64], F32, kind="Internal",
                            addr_space="Shared")
    g2_sl = nc.dram_tensor("g2_slice", [NWS, 64], F32, kind="Internal")
    g2_full = nc.dram_tensor("g2_full", [PAD_N, 64], F32, kind="Internal",
                             addr_space="Shared")
    g3_sl = nc.dram_tensor("g3_slice", [NWS, 64], F32, kind="Internal")
    g3_full = nc.dram_tensor("g3_full", [PAD_N, 64], F32, kind="Internal",
                             addr_space="Shared")
    pool_in = nc.dram_tensor("pool_in", [GP, 64], F32, kind="Internal")
    pool_out = nc.dram_tensor("pool_out", [GP, 64], F32, kind="Internal",
                              addr_space="Shared")

    with tile.TileContext(nc) as tc, ExitStack() as ctx:
        P = ctx.enter_context
        setup = P(tc.tile_pool(name="setup", bufs=1))
        oh_pool = P(tc.tile_pool(name="oh", bufs=3))
        gb_pool = P(tc.tile_pool(name="gb", bufs=3 if cfg.GSZ <= 4 else 2))
        fn_pool = P(tc.tile_pool(name="fn", bufs=2))
        psS = P(tc.tile_pool(name="psS", bufs=2, space="PSUM"))
        psZ = P(tc.tile_pool(name="psZ", bufs=2, space="PSUM"))
        psT = P(tc.tile_pool(name="psT", bufs=2, space="PSUM"))
        psHold = P(tc.tile_pool(name="psHold", bufs=1, space="PSUM"))
        ev1 = P(tc.tile_pool(name="ev1", bufs=3))
        ev2 = P(tc.tile_pool(name="ev2", bufs=3))
        ev3 = P(tc.tile_pool(name="ev3", bufs=3))
        stg = P(tc.tile_pool(name="stg", bufs=1))

        def load(name, shape, dt=F32):
            t = setup.tile(list(shape), dt, tag=name)
            nc.sync.dma_start(out=t[:], in_=din[name].ap()[:])
            return t

        ixa = load("ixa", (128, TOT_A), I16)
        ixb = load("ixb", (128, TOT_B), I16)
        dstrel = load("ei_dstrel", (128, NW * NT))
        nd_deg = load("nd_deg", (128, NW))
        nd_x = load("nd_x", (128, NW))
        nd_batch = load("nd_batch", (128, NW))
        degT = load("degT", (128, NWS))
        iota = load("iota", (128, GP))
        cnts = load("cnts", (128, 2))
        w1 = load("w1", (1, 64)); w2 = load("w2", (64, 128))
        w3 = load("w3", (128, 64))
        b1 = load("b1", (64, 1)); b2 = load("b2", (128, 1))
        b3 = load("b3", (64, 1))
        l1w = load("l1w", (64, 32)); l1b = load("l1b", (32, 1))
        l2w = load("l2w", (32, 1)); l2b = load("l2b", (128, 1))

        ident = setup.tile([128, 128], F32, tag="ident")
        make_identity(nc, ident[:])
        ones_col = setup.tile([128, 1], F32, tag="ones")
        nc.vector.memset(ones_col[:], 1.0)
        fence_ix = setup.tile([128, 8], I16, tag="fence_ix")
        nc.vector.memset(fence_ix[:], 0)

        dinvT = setup.tile([128, NWS], F32, tag="dinvT")
        nc.scalar.activation(out=dinvT[:], in_=degT[:], func=AF.Sqrt,
                             bias=1.0, scale=1.0)
        nc.vector.reciprocal(out=dinvT[:], in_=dinvT[:])
        dinv_nm = setup.tile([128, NW], F32, tag="dinv_nm")
        nc.scalar.activation(out=dinv_nm[:], in_=nd_deg[:], func=AF.Sqrt,
                             bias=1.0, scale=1.0)
        nc.vector.reciprocal(out=dinv_nm[:], in_=dinv_nm[:])
        y_nm = setup.tile([128, NW], F32, tag="y_nm")
        nc.vector.tensor_tensor(out=y_nm[:], in0=nd_x[:], in1=dinv_nm[:],
                                op=OP.mult)

        staging = stg.tile([128, NW * 64], F32, tag="staging")

        # y table: y replicated to 64 lanes, node-major
        nc.vector.tensor_copy(
            out=staging[:].rearrange("p (w f) -> p w f", f=64),
            in_=y_nm[:, :, None].to_broadcast([128, NW, 64]))
        nc.sync.dma_start(
            out=y_sl.ap()[:].rearrange("(w p) f -> p w f", p=128),
            in_=staging[:].rearrange("p (w f) -> p w f", f=64))
        if no_coll:
            nc.gpsimd.dma_start(out=y_full.ap()[:NWS, :],
                                in_=y_sl.ap()[:])
        else:
            nc.gpsimd.collective_compute(
                "AllGather", OP.bypass, replica_groups=rg,
                ins=[y_sl.ap()[:]], outs=[y_full.ap()[:]])

        def gather_group(gi, gn, table):
            """fenced grouped gather; returns (gb_tile, fence_inst)."""
            gb = gb_pool.tile([128, cfg.GSZ * NT * 64], F32, tag="gb")
            nA, nB = gn * NT_A * 128, gn * NT_B * 128
            callA = nc.gpsimd.dma_gather(
                out_ap=gb[:, :nA // 128 * 64].rearrange(
                    "p (t f) -> p t f", f=64),
                in_ap=table.ap()[:HALF, :],
                idxs_ap=ixa[:, offA[gi]:offA[gi + 1]],
                num_idxs=nA, num_idxs_reg=nA, elem_size=64,
                single_packet=False)
            callB = nc.gpsimd.dma_gather(
                out_ap=gb[:, nA // 128 * 64:(nA + nB) // 128 * 64].rearrange(
                    "p (t f) -> p t f", f=64),
                in_ap=table.ap()[HALF:, :],
                idxs_ap=ixb[:, offB[gi]:offB[gi + 1]],
                num_idxs=nB, num_idxs_reg=nB, elem_size=64,
                single_packet=False)
            fence_t = fn_pool.tile([128, 64], F32, tag="fence")
            fence = nc.gpsimd.dma_gather(
                out_ap=fence_t[:].rearrange("p (t f) -> p t f", f=64),
                in_ap=table.ap()[:HALF, :],
                idxs_ap=fence_ix[:],
                num_idxs=128, num_idxs_reg=128, elem_size=64,
                single_packet=True)
            add_dep_helper(fence.ins, callA.ins, True, "fence>A")
            add_dep_helper(fence.ins, callB.ins, True, "fence>B")
            return gb, fence

        def onehot_win(w):
            oh = oh_pool.tile([128, NT * 128], F32, tag="oh")
            dr3 = dstrel[:, w * NT:(w + 1) * NT][:, :, None].to_broadcast(
                [128, NT, 128])
            io3 = iota[:, None, :128].to_broadcast([128, NT, 128])
            nc.vector.tensor_tensor(
                out=oh[:].rearrange("p (t j) -> p t j", j=128),
                in0=dr3, in1=io3, op=OP.is_equal)
            return oh

        def scatter_win(oh, gb, fence, wl, gn, F):
            ps = psS.tile([F, 128], F32, space="PSUM", tag="psS")
            for t in range(NT):
                c = chunk_of(cfg, wl, t, gn)
                mm = nc.tensor.matmul(
                    out=ps[:], lhsT=gb[:, c * 64:c * 64 + F],
                    rhs=oh[:, t * 128:(t + 1) * 128],
                    start=(t == 0), stop=(t == NT - 1))
                add_dep_helper(mm.ins, fence.ins, True, "mm>fence")
            return ps

        def wsl(w):
            return slice(w * 128, (w + 1) * 128)

        for _rep in range(reps):
            # ---- Layer 1 --------------------------------------------------------
            for gi, (ws, gn) in enumerate(cfg.groups):
                gb, fence = gather_group(gi, gn, y_full)
                for wl in range(gn):
                    w = ws + wl
                    oh = onehot_win(w)
                    ps1 = scatter_win(oh, gb, fence, wl, gn, 1)
                    s1 = ev1.tile([1, 128], F32, tag="s1")
                    nc.scalar.activation(out=s1[:], in_=ps1[:], func=AF.Copy)
                    psO = psZ.tile([64, 128], F32, space="PSUM", tag="psz")
                    nc.tensor.matmul(out=psO[:], lhsT=w1[:], rhs=s1[:],
                                     start=True, stop=True)
                    m1 = ev2.tile([64, 128], F32, tag="m1")
                    nc.vector.tensor_tensor(out=m1[:], in0=psO[:],
                                            in1=dinvT[:64, wsl(w)], op=OP.mult)
                    h1 = ev3.tile([64, 128], F32, tag="h1")
                    nc.scalar.activation(out=h1[:], in_=m1[:], func=AF.Relu,
                                         bias=b1[:])
                    g2 = ev2.tile([64, 128], F32, tag="g2")
                    nc.vector.tensor_tensor(out=g2[:], in0=h1[:],
                                            in1=dinvT[:64, wsl(w)], op=OP.mult)
                    psN = psT.tile([128, 64], F32, space="PSUM", tag="psN")
                    nc.tensor.transpose(out=psN[:], in_=g2[:],
                                        identity=ident[:64, :64])
                    nc.scalar.activation(out=staging[:, w * 64:(w + 1) * 64],
                                         in_=psN[:], func=AF.Copy)

            nc.sync.dma_start(
                out=g2_sl.ap()[:].rearrange("(w p) f -> p w f", p=128),
                in_=staging[:].rearrange("p (w f) -> p w f", f=64))
            nc.gpsimd.collective_compute(
                "AllGather", OP.bypass, replica_groups=rg,
                ins=[g2_sl.ap()[:]], outs=[g2_full.ap()[:]])

            # ---- Layer 2 (+ fold W3, produce g3) --------------------------------
            for gi, (ws, gn) in enumerate(cfg.groups):
                gb, fence = gather_group(gi, gn, g2_full)
                for wl in range(gn):
                    w = ws + wl
                    oh = onehot_win(w)
                    ps2 = scatter_win(oh, gb, fence, wl, gn, 64)
                    aggT = ev1.tile([64, 128], F32, tag="aggT")
                    nc.vector.tensor_tensor(out=aggT[:], in0=ps2[:],
                                            in1=dinvT[:64, wsl(w)], op=OP.mult)
                    psz = psZ.tile([128, 128], F32, space="PSUM", tag="psz")
                    nc.tensor.matmul(out=psz[:], lhsT=w2[:], rhs=aggT[:],
                                     start=True, stop=True)
                    h2 = ev2.tile([128, 128], F32, tag="h2")
                    nc.scalar.activation(out=h2[:], in_=psz[:], func=AF.Relu,
                                         bias=b2[:])
                    pst3 = psZ.tile([64, 128], F32, space="PSUM", tag="psz")
                    nc.tensor.matmul(out=pst3[:], lhsT=w3[:], rhs=h2[:],
                                     start=True, stop=True)
                    g3 = ev3.tile([64, 128], F32, tag="g3")
                    nc.vector.tensor_tensor(out=g3[:], in0=pst3[:],
                                            in1=dinvT[:64, wsl(w)], op=OP.mult)
                    psN = psT.tile([128, 64], F32, space="PSUM", tag="psN")
                    nc.tensor.transpose(out=psN[:], in_=g3[:],
                                        identity=ident[:64, :64])
                    nc.scalar.activation(out=staging[:, w * 64:(w + 1) * 64],
                                         in_=psN[:], func=AF.Copy)

            nc.sync.dma_start(
                out=g3_sl.ap()[:].rearrange("(w p) f -> p w f", p=128),
                in_=staging[:].rearrange("p (w f) -> p w f", f=64))
            nc.gpsimd.collective_compute(
                "AllGather", OP.bypass, replica_groups=rg,
                ins=[g3_sl.ap()[:]], outs=[g3_full.ap()[:]])

            # ---- Layer 3 + pooling ----------------------------------------------
            pooled_a = psHold.tile([128, 64], F32, space="PSUM", tag="pool_a")
            pooled_b = psHold.tile([128, 64], F32, space="PSUM", tag="pool_b")
            for gi, (ws, gn) in enumerate(cfg.groups):
                gb, fence = gather_group(gi, gn, g3_full)
                for wl in range(gn):
                    w = ws + wl
                    oh = onehot_win(w)
                    ps3 = scatter_win(oh, gb, fence, wl, gn, 64)
                    agg3 = ev1.tile([64, 128], F32, tag="aggT")
                    nc.vector.tensor_tensor(out=agg3[:], in0=ps3[:],
                                            in1=dinvT[:64, wsl(w)], op=OP.mult)
                    h3 = ev2.tile([64, 128], F32, tag="h3")
                    nc.scalar.activation(out=h3[:], in_=agg3[:], func=AF.Relu,
                                         bias=b3[:])
                    psN = psT.tile([128, 64], F32, space="PSUM", tag="psN")
                    nc.tensor.transpose(out=psN[:], in_=h3[:],
                                        identity=ident[:64, :64])
                    h3nm = ev3.tile([128, 64], F32, tag="h3nm")
                    nc.scalar.activation(out=h3nm[:], in_=psN[:], func=AF.Copy)
                    ohp = oh_pool.tile([128, GP], F32, tag="ohp")
                    bc = nd_batch[:, w:w + 1].to_broadcast([128, GP])
                    nc.vector.tensor_tensor(out=ohp[:], in0=bc, in1=iota[:],
                                            op=OP.is_equal)
                    for half, ps_pool in ((0, pooled_a), (1, pooled_b)):
                        lhs = ohp[:, half * 128:(half + 1) * 128]
                        nc.tensor.matmul(out=ps_pool[:], lhsT=lhs,
                                         rhs=h3nm[:],
                                         start=(w == 0), stop=(w == NW - 1))

            # ---- finale ---------------------------------------------------------
            pa = setup.tile([128, 64], F32, tag="pa")
            pb = setup.tile([128, 64], F32, tag="pb")
            nc.scalar.activation(out=pa[:], in_=pooled_a[:], func=AF.Copy)
            nc.scalar.activation(out=pb[:], in_=pooled_b[:], func=AF.Copy)
            nc.sync.dma_start(out=pool_in.ap()[0:128, :], in_=pa[:])
            nc.sync.dma_start(out=pool_in.ap()[128:256, :], in_=pb[:])
            if no_coll:
                nc.gpsimd.dma_start(out=pool_out.ap()[:],
                                    in_=pool_in.ap()[:])
            else:
                nc.gpsimd.collective_compute(
                    "AllReduce", OP.add, replica_groups=rg,
                    ins=[pool_in.ap()[:]], outs=[pool_out.ap()[:]])

        meanT = setup.tile([64, 256], F32, tag="meanT")
        for half in (0, 1):
            pl = setup.tile([128, 64], F32, tag=f"pl{half}")
            nc.sync.dma_start(
                out=pl[:], in_=pool_out.ap()[half * 128:(half + 1) * 128, :])
            cntm = setup.tile([128, 1], F32, tag=f"cntm{half}")
            nc.vector.tensor_scalar_max(out=cntm[:],
                                        in0=cnts[:, half:half + 1],
                                        scalar1=1.0)
            rc = setup.tile([128, 1], F32, tag=f"rc{half}")
            nc.vector.reciprocal(out=rc[:], in_=cntm[:])
            mean = setup.tile([128, 64], F32, tag=f"mean{half}")
            nc.vector.tensor_scalar_mul(out=mean[:], in0=pl[:],
                                        scalar1=rc[:])
            psMT = psT.tile([64, 128], F32, space="PSUM", tag="psN")
            nc.tensor.transpose(out=psMT[:], in_=mean[:], identity=ident[:])
            nc.scalar.activation(out=meanT[:, half * 128:(half + 1) * 128],
                                 in_=psMT[:], func=AF.Copy)

        psZ1 = psZ.tile([32, 256], F32, space="PSUM", tag="psz")
        nc.tensor.matmul(out=psZ1[:], lhsT=l1w[:], rhs=meanT[:],
                         start=True, stop=True)
        z1 = setup.tile([32, 256], F32, tag="z1")
        nc.scalar.activation(out=z1[:], in_=psZ1[:], func=AF.Relu,
                             bias=l1b[:])
        for half in (0, 1):
            psO = psT.tile([128, 1], F32, space="PSUM", tag="psN")
            nc.tensor.matmul(out=psO[:],
                             lhsT=z1[:, half * 128:(half + 1) * 128],
                             rhs=l2w[:], start=True, stop=True)
            ob = setup.tile([128, 1], F32, tag=f"ob{half}")
            nc.scalar.activation(out=ob[:], in_=psO[:], func=AF.Identity,
                                 bias=l2b[:])
            nc.sync.dma_start(out=out_d.ap()[half * 128:(half + 1) * 128, :],
                              in_=ob[:])

    nc.compile()
    return nc


# ----------------------------------------------------------------------------
# Runner
# ----------------------------------------------------------------------------

_CACHE = {}


def get_program(cfg, meta, reps=1, no_coll=False):
    TOT_A, TOT_B, offA, offB = meta
    key = cfg.key() + (reps, no_coll)
    if key not in _CACHE:
        _CACHE[key] = build_program(cfg, TOT_A, TOT_B, offA, offB, reps,
                                    no_coll)
    return _CACHE[key]


def run(cfg, inputs, trace=False):
    in_maps, meta = prep_inputs(cfg, **inputs)
    nc = get_program(cfg, meta)
    res = bass_utils.run_bass_kernel_spmd(
        nc, in_maps, core_ids=list(range(cfg.C)), trace=trace)
    out = res.results[0]["out"][:cfg.G, :].astype(np.float32)
    return out, res


def make_cfg(inputs, n_nodes=50000, n_edges=600000, n_graphs=256,
             nw=49, half=32768, gsz=GSZ):
    proto = Cfg(n_nodes, n_edges, n_graphs, 8, nw, 1, 1, half, gsz)
    nt_a, nt_b = compute_caps(n_nodes, inputs["edge_index"], proto)
    return Cfg(n_nodes, n_edges, n_graphs, 8, nw, nt_a, nt_b, half, gsz)


def kernel(**inputs) -> np.ndarray:
    cfg = make_cfg(inputs)
    out, _ = run(cfg, inputs)
    return out



# revision 12
# speedup vs baseline: 29.4313x; 29.4313x over previous
"""GCN binding-affinity model on 8 TRN2 cores — v2.

Structural changes vs v1 (kernel.py):
  * L1 aggregation needs NO device gather: per-edge x[src] / deg[src] are
    shipped as host-sharded edge data (input sharding), normalized and
    scatter-summed on device (F=1 one-hot matmuls).
  * h1 = relu(outer(a, w1)) is exactly rank 2 (b1 == 0 per problem spec):
    h1 = relu(a) w1+ + relu(-a) w1-.  So L2's message table is just the two
    scalars s+/- = dinv * relu(+-a) per node -> AllGather 400KB instead of
    12.8MB; the 64-wide expansion happens AFTER aggregation via the fixed
    vectors u = relu(w1)@W2, v = relu(-w1)@W2.
  * Self-loop contributions are applied locally (not as gathered edges).
  * Exact per-window tile packing (variable ntA/ntB per window) instead of
    global caps -> ~20% fewer gather descriptors / matmuls / one-hots.
  * Gather tables are [*, 64] f32 rows (256B, the SWDGE minimum); for the
    scalar L2 table only cols 0:2 hold data (lhsT never reads the rest).

Math identical to reference:
  per layer: agg = dinv (.) ((A+I) (dinv (.) t)), dinv = rsqrt(indeg+1).
  L1 t = x (scalar), L2 t = h1 (rank 2 -> 2 scalar channels), L3 t = h2@W3.
"""

import os
import sys
from contextlib import ExitStack

import numpy as np

for _p in ("/opt/trn_rl_repo",):
    if _p not in sys.path and os.path.isdir(_p):
        sys.path.insert(0, _p)

import concourse.bass as bass
import concourse.mybir as mybir
import concourse.tile as tile
from concourse import bacc
from concourse import bass_utils
from concourse.masks import make_identity
from concourse.tile_rust import add_dep_helper

F32 = mybir.dt.float32
I16 = mybir.dt.int16
AF = mybir.ActivationFunctionType
OP = mybir.AluOpType

N_NODES = 50000
N_EDGES = 600000
N_GRAPHS = 256
C = 8
NW = 49
NWS = NW * 128          # 6272 nodes per core
PAD_N = C * NWS         # 50176
HALF = 32768
GP = 256
GSZ = 4                 # windows per gather group


def wrap16(vals):
    return np.ascontiguousarray(vals.reshape(-1, 16).T.astype(np.int16))


def rep8(block):
    return np.tile(block, (8, 1))


class Plan:
    """Per-core edge packing metadata (topology-derived, shared across cores
    as parallel lists)."""

    def __init__(self, edge_index):
        src = np.asarray(edge_index[0], dtype=np.int64)
        dst = np.asarray(edge_index[1], dtype=np.int64)
        self.deg = np.bincount(dst, minlength=N_NODES).astype(np.float32)

        core_of = dst // NWS
        order0 = np.argsort(core_of, kind="stable")
        src, dst, core_sorted = src[order0], dst[order0], core_of[order0]

        self.cores = []
        for c in range(C):
            base = c * NWS
            lo = np.searchsorted(core_sorted, c, side="left")
            hi = np.searchsorted(core_sorted, c, side="right")
            c_src, c_dst = src[lo:hi], dst[lo:hi]
            w_of = (c_dst - base) >> 7
            is_a = c_src < HALF
            key = w_of * 2 + (~is_a)
            order = np.argsort(key, kind="stable")
            c_src, c_dst, w_of, is_a, key = (
                c_src[order], c_dst[order], w_of[order], is_a[order],
                key[order])
            kstart = np.searchsorted(key, np.arange(2 * NW), side="left")
            kend = np.searchsorted(key, np.arange(2 * NW), side="right")
            cnt = kend - kstart
            cntA, cntB = cnt[0::2], cnt[1::2]
            ntA = np.ceil(cntA / 128).astype(int)
            ntB = np.ceil(cntB / 128).astype(int)
            nt = ntA + ntB
            rank = np.arange(len(key)) - kstart[key]  # rank within (w, half)

            tbase = np.concatenate([[0], np.cumsum(nt)]).astype(int)
            T_total = int(tbase[-1])

            # groups
            groups = []
            offA = [0]
            offB = [0]
            for ws in range(0, NW, GSZ):
                gn = min(GSZ, NW - ws)
                wl = np.arange(ws, ws + gn)
                gA = int(ntA[wl].sum())
                gB = int(ntB[wl].sum())
                cA0 = np.concatenate([[0], np.cumsum(ntA[wl])]).astype(int)
                cB0 = np.concatenate([[0], np.cumsum(ntB[wl])]).astype(int)
                groups.append(dict(ws=ws, gn=gn, gA=gA, gB=gB,
                                   cA0=cA0, cB0=cB0,
                                   ntA=ntA[wl].copy(), ntB=ntB[wl].copy(),
                                   colA=offA[-1], colB=offB[-1]))
                offA.append(offA[-1] + gA * 128 // 16)
                offB.append(offB[-1] + gB * 128 // 16)
            TOT_A, TOT_B = offA[-1], offB[-1]

            # per-edge global tile index + lane
            gt = np.where(
                is_a,
                tbase[w_of] + rank // 128,
                tbase[w_of] + ntA[w_of] + rank // 128)
            lane = rank % 128

            # per-edge position within the group idx sequences
            gi_of = w_of // GSZ
            grpA_off = np.zeros(NW, int)
            grpB_off = np.zeros(NW, int)
            for g in groups:
                ws, gn = g["ws"], g["gn"]
                grpA_off[ws:ws + gn] = g["cA0"][:gn] * 128
                grpB_off[ws:ws + gn] = g["cB0"][:gn] * 128
            colA_base = np.array([g["colA"] * 16 for g in groups])
            colB_base = np.array([g["colB"] * 16 for g in groups])
            posA = colA_base[gi_of] + grpA_off[w_of] + rank
            posB = colB_base[gi_of] + grpB_off[w_of] + rank

            seqA = np.zeros(TOT_A * 16, np.int64)
            seqB = np.zeros(TOT_B * 16, np.int64)
            seqA[posA[is_a]] = c_src[is_a]
            seqB[posB[~is_a]] = c_src[~is_a] - HALF

            self.cores.append(dict(
                base=base, src=c_src, dst=c_dst, w_of=w_of, is_a=is_a,
                rank=rank, ntA=ntA, ntB=ntB, nt=nt, tbase=tbase,
                T_total=T_total, groups=groups, TOT_A=TOT_A, TOT_B=TOT_B,
                gt=gt, lane=lane, seqA=seqA, seqB=seqB))

        self.maxT = max(cc["T_total"] for cc in self.cores)
        self.maxNT = max(int(cc["nt"].max()) for cc in self.cores)
        self.maxTOT_A = max(cc["TOT_A"] for cc in self.cores)
        self.maxTOT_B = max(cc["TOT_B"] for cc in self.cores)
        self.max_gC = max(g["gA"] + g["gB"]
                          for cc in self.cores for g in cc["groups"])
        # the device program is built once from core 0's plan shape; all
        # cores must share the same unrolled structure -> pad counts to the
        # max across cores.  Instead of that complexity: build per-core
        # programs?  SPMD needs ONE program.  So we equalize the plan:
        self._equalize()

    def _equalize(self):
        """Pad every core's per-window tile counts up to the max across
        cores so a single SPMD program fits all cores."""
        ntA_max = np.max([cc["ntA"] for cc in self.cores], axis=0)
        ntB_max = np.max([cc["ntB"] for cc in self.cores], axis=0)
        nt = ntA_max + ntB_max
        tbase = np.concatenate([[0], np.cumsum(nt)]).astype(int)
        T_total = int(tbase[-1])
        groups = []
        offA = [0]
        offB = [0]
        for ws in range(0, NW, GSZ):
            gn = min(GSZ, NW - ws)
            wl = np.arange(ws, ws + gn)
            gA = int(ntA_max[wl].sum())
            gB = int(ntB_max[wl].sum())
            cA0 = np.concatenate([[0], np.cumsum(ntA_max[wl])]).astype(int)
            cB0 = np.concatenate([[0], np.cumsum(ntB_max[wl])]).astype(int)
            groups.append(dict(ws=ws, gn=gn, gA=gA, gB=gB, cA0=cA0, cB0=cB0,
                               ntA=ntA_max[wl].copy(), ntB=ntB_max[wl].copy(),
                               colA=offA[-1], colB=offB[-1]))
            offA.append(offA[-1] + gA * 128 // 16)
            offB.append(offB[-1] + gB * 128 // 16)
        TOT_A, TOT_B = offA[-1], offB[-1]

        grpA_off = np.zeros(NW, int)
        grpB_off = np.zeros(NW, int)
        colA_base = np.zeros(NW, int)
        colB_base = np.zeros(NW, int)
        for g in groups:
            ws, gn = g["ws"], g["gn"]
            grpA_off[ws:ws + gn] = g["cA0"][:gn] * 128
            grpB_off[ws:ws + gn] = g["cB0"][:gn] * 128
            colA_base[ws:ws + gn] = g["colA"] * 16
            colB_base[ws:ws + gn] = g["colB"] * 16

        for cc in self.cores:
            w_of, is_a, rank = cc["w_of"], cc["is_a"], cc["rank"]
            gt = np.where(is_a,
                          tbase[w_of] + rank // 128,
                          tbase[w_of] + ntA_max[w_of] + rank // 128)
            posA = colA_base[w_of] + grpA_off[w_of] + rank
            posB = colB_base[w_of] + grpB_off[w_of] + rank
            seqA = np.zeros(TOT_A * 16, np.int64)
            seqB = np.zeros(TOT_B * 16, np.int64)
            seqA[posA[is_a]] = cc["src"][is_a]
            seqB[posB[~is_a]] = cc["src"][~is_a] - HALF
            cc.update(gt=gt, seqA=seqA, seqB=seqB)

        self.ntA = ntA_max
        self.ntB = ntB_max
        self.nt = nt
        self.tbase = tbase
        self.T_total = T_total
        self.groups = groups
        self.TOT_A = TOT_A
        self.TOT_B = TOT_B
        self.max_gC = max(g["gA"] + g["gB"] for g in groups)

    def key(self):
        return (self.T_total, self.TOT_A, self.TOT_B,
                tuple(self.nt.tolist()))


def prep_inputs(plan, x, W1, b1, W2, b2, W3, b3, lin1_w, lin1_b, lin2_w,
                lin2_b, edge_index, batch):
    assert not np.any(np.asarray(b1)), "v2 exploits b1 == 0 (rank-2 h1)"
    assert not np.any(np.asarray(b2)), "t3local exploits b2 == 0"
    x = np.asarray(x, np.float32).reshape(-1)
    batch = np.asarray(batch, dtype=np.int64)
    deg = plan.deg

    x_ext = np.zeros(PAD_N, np.float32); x_ext[:N_NODES] = x
    deg_ext = np.zeros(PAD_N, np.float32); deg_ext[:N_NODES] = deg
    batch_ext = np.full(PAD_N, -1.0, np.float32)
    batch_ext[:N_NODES] = batch.astype(np.float32)

    iota = np.broadcast_to(np.arange(GP, dtype=np.float32),
                           (128, GP)).copy()
    cnts = np.bincount(batch, minlength=GP).astype(np.float32)
    cnts2 = np.ascontiguousarray(cnts.reshape(2, 128).T)

    in_maps = []
    for c, cc in enumerate(plan.cores):
        T = plan.T_total
        drel = np.full((128, T), -1.0, np.float32)
        ysx = np.zeros((128, T), np.float32)
        ysd = np.zeros((128, T), np.float32)
        gt, lane = cc["gt"], cc["lane"]
        drel[lane, gt] = (cc["dst"] - cc["base"] - (cc["w_of"] << 7)
                          ).astype(np.float32)
        ysx[lane, gt] = x[cc["src"]]
        ysd[lane, gt] = deg[cc["src"]]

        ixa = rep8(wrap16(cc["seqA"]))
        ixb = rep8(wrap16(cc["seqB"]))

        base = cc["base"]
        sl = slice(base, base + NWS)
        nd = lambda a: np.ascontiguousarray(a[sl].reshape(NW, 128).T)
        degT = np.broadcast_to(deg_ext[sl][None, :], (128, NWS)).copy()

        in_maps.append({
            "ixa": ixa, "ixb": ixb,
            "drel": drel, "ysx": ysx, "ysd": ysd,
            "nd_batch": nd(batch_ext),
            "nd_x": nd(x_ext), "nd_deg": nd(deg_ext),
            "degT": degT,
            "iota": iota, "cnts": cnts2,
            "w1": np.asarray(W1, np.float32).reshape(1, 64),
            "w2": np.asarray(W2, np.float32).reshape(64, 128),
            "w3": np.asarray(W3, np.float32).reshape(128, 64),
            "b2": np.asarray(b2, np.float32).reshape(128, 1),
            "b3": np.asarray(b3, np.float32).reshape(64, 1),
            "l1w": np.asarray(lin1_w, np.float32).reshape(64, 32),
            "l1b": np.asarray(lin1_b, np.float32).reshape(32, 1),
            "l2w": np.asarray(lin2_w, np.float32).reshape(32, 1),
            "l2b": np.full((128, 1),
                           np.float32(np.asarray(lin2_b).reshape(())),
                           np.float32),
        })
    return in_maps


# ----------------------------------------------------------------------------
# Device program
# ----------------------------------------------------------------------------

def build_program(plan, reps=1, no_coll=False, no_gather=False,
                  no_mm=False, no_oh=False, t3local=False):
    rg = [list(range(C))]
    T = plan.T_total
    TOT_A, TOT_B = plan.TOT_A, plan.TOT_B
    MAXNT = int(plan.nt.max())
    MAXGC = plan.max_gC

    nc = bacc.Bacc("TRN2", target_bir_lowering=False, debug=False,
                   num_devices=C)

    din = {}
    def inp(name, shape, dt=F32):
        din[name] = nc.dram_tensor(name, list(shape), dt,
                                   kind="ExternalInput")
        return din[name]

    inp("ixa", (128, TOT_A), I16)
    inp("ixb", (128, TOT_B), I16)
    inp("drel", (128, T)); inp("ysx", (128, T)); inp("ysd", (128, T))
    inp("nd_batch", (128, NW))
    inp("nd_x", (128, NW)); inp("nd_deg", (128, NW))
    inp("degT", (128, NWS))
    inp("iota", (128, GP)); inp("cnts", (128, 2))
    inp("w1", (1, 64)); inp("w2", (64, 128)); inp("w3", (128, 64))
    inp("b2", (128, 1)); inp("b3", (64, 1))
    inp("l1w", (64, 32)); inp("l1b", (32, 1)); inp("l2w", (32, 1))
    inp("l2b", (128, 1))

    out_d = nc.dram_tensor("out", [GP, 1], F32, kind="ExternalOutput")

    t2c_sl = nc.dram_tensor("t2c_slice", [NWS, 2], F32, kind="Internal")
    t2c_full = nc.dram_tensor("t2c_full", [PAD_N, 2], F32, kind="Internal",
                              addr_space="Shared")
    t2 = nc.dram_tensor("t2", [PAD_N, 64], F32, kind="Internal")
    y3_sl = nc.dram_tensor("y3_slice", [NWS, 64], F32, kind="Internal")
    y3_full = nc.dram_tensor("y3_full", [PAD_N, 64], F32, kind="Internal",
                             addr_space="Shared")
    t3c_sl = nc.dram_tensor("t3c_slice", [2, NWS], F32, kind="Internal")
    t3c_full = nc.dram_tensor("t3c_full", [2 * C, NWS], F32, kind="Internal",
                              addr_space="Shared")
    t3 = nc.dram_tensor("t3", [PAD_N, 64], F32, kind="Internal")
    pool_in = nc.dram_tensor("pool_in", [GP, 64], F32, kind="Internal")
    pool_out = nc.dram_tensor("pool_out", [GP, 64], F32, kind="Internal",
                              addr_space="Shared")

    with tile.TileContext(nc) as tc, ExitStack() as ctx:
        P = ctx.enter_context
        setup = P(tc.tile_pool(name="setup", bufs=1))
        oh_pool = P(tc.tile_pool(name="oh", bufs=2))
        gb_pool = P(tc.tile_pool(name="gb", bufs=2))
        fn_pool = P(tc.tile_pool(name="fn", bufs=2))
        psS = P(tc.tile_pool(name="psS", bufs=2, space="PSUM"))
        psZ = P(tc.tile_pool(name="psZ", bufs=2, space="PSUM"))
        psT = P(tc.tile_pool(name="psT", bufs=2, space="PSUM"))
        psHold = P(tc.tile_pool(name="psHold", bufs=1, space="PSUM"))
        ev1 = P(tc.tile_pool(name="ev1", bufs=3))
        ev2 = P(tc.tile_pool(name="ev2", bufs=3))
        ev3 = P(tc.tile_pool(name="ev3", bufs=3))
        stg = P(tc.tile_pool(name="stg", bufs=1))
        expp = P(tc.tile_pool(name="expp", bufs=2))

        def load(name, shape, dt=F32):
            t = setup.tile(list(shape), dt, tag=name)
            nc.sync.dma_start(out=t[:], in_=din[name].ap()[:])
            return t

        ixa = load("ixa", (128, TOT_A), I16)
        ixb = load("ixb", (128, TOT_B), I16)
        drel = load("drel", (128, T))
        ysx = load("ysx", (128, T))
        ysd = load("ysd", (128, T))
        nd_batch = load("nd_batch", (128, NW))
        nd_x = load("nd_x", (128, NW))
        nd_deg = load("nd_deg", (128, NW))
        degT = load("degT", (128, NWS))
        iota = load("iota", (128, GP))
        cnts = load("cnts", (128, 2))
        w1 = load("w1", (1, 64)); w2 = load("w2", (64, 128))
        w3 = load("w3", (128, 64))
        b2 = load("b2", (128, 1)); b3 = load("b3", (64, 1))
        l1w = load("l1w", (64, 32)); l1b = load("l1b", (32, 1))
        l2w = load("l2w", (32, 1)); l2b = load("l2b", (128, 1))

        ident = setup.tile([128, 128], F32, tag="ident")
        make_identity(nc, ident[:])
        fence_ix = setup.tile([128, 8], I16, tag="fence_ix")
        nc.vector.memset(fence_ix[:], 0)

        # dinvT = rsqrt(degT + 1) computed in place (free dim layout)
        dinvT = degT
        nc.scalar.activation(out=dinvT[:], in_=degT[:], func=AF.Sqrt,
                             bias=1.0, scale=1.0)
        nc.vector.reciprocal(out=dinvT[:], in_=dinvT[:])

        # per-slot src normalization: ys = x[src] * rsqrt(deg[src]+1)
        nc.scalar.activation(out=ysd[:], in_=ysd[:], func=AF.Sqrt,
                             bias=1.0, scale=1.0)
        nc.vector.reciprocal(out=ysd[:], in_=ysd[:])
        ys = ysx
        nc.vector.tensor_tensor(out=ys[:], in0=ysd[:], in1=ysx[:],
                                op=OP.mult)

        # own-node x*dinv, node-partition layout (L1 self-loop lhsT)
        dinv_nm = setup.tile([128, NW], F32, tag="dinv_nm")
        nc.scalar.activation(out=dinv_nm[:], in_=nd_deg[:], func=AF.Sqrt,
                             bias=1.0, scale=1.0)
        nc.vector.reciprocal(out=dinv_nm[:], in_=dinv_nm[:])
        xd_nm = setup.tile([128, NW], F32, tag="xd_nm")
        nc.vector.tensor_tensor(out=xd_nm[:], in0=nd_x[:], in1=dinv_nm[:],
                                op=OP.mult)

        # UVT = [u; v] = [relu(w1); relu(-w1)] @ W2  -> [2, 128]
        w1p = setup.tile([1, 64], F32, tag="w1p")
        w1m = setup.tile([1, 64], F32, tag="w1m")
        nc.scalar.activation(out=w1p[:], in_=w1[:], func=AF.Relu)
        nc.scalar.activation(out=w1m[:], in_=w1[:], func=AF.Relu,
                             scale=-1.0)
        w1pmT = setup.tile([64, 2], F32, tag="w1pmT")
        for i, src_t in enumerate((w1p, w1m)):
            psx = psT.tile([128, 128], F32, space="PSUM", tag="psN")
            nc.tensor.transpose(out=psx[:64, :1], in_=src_t[:],
                                identity=ident[:1, :1])
            nc.scalar.activation(out=w1pmT[:, i:i + 1], in_=psx[:64, :1],
                                 func=AF.Copy)
        psUV = psZ.tile([128, 256], F32, space="PSUM", tag="psz")
        nc.tensor.matmul(out=psUV[:2, :128], lhsT=w1pmT[:], rhs=w2[:],
                         start=True, stop=True)
        UVT = setup.tile([2, 128], F32, tag="UVT")
        nc.scalar.activation(out=UVT[:], in_=psUV[:2, :128], func=AF.Copy)

        staging = stg.tile([128, NW * 64], F32, tag="staging")
        staging2 = stg.tile([128, NW * 2], F32, tag="staging2")
        if t3local:
            staging2c = stg.tile([2, NWS], F32, tag="staging2c")
        else:
            staging2c = None

        def wsl(w):
            return slice(w * 128, (w + 1) * 128)

        def onehot_win(w):
            ntw = int(plan.nt[w])
            t0 = int(plan.tbase[w])
            oh = oh_pool.tile([128, MAXNT * 128], F32, tag="oh")
            if no_oh or ntw == 0:
                nc.vector.memset(oh[:], 0.0)
                return oh
            dr3 = drel[:, t0:t0 + ntw][:, :, None].to_broadcast(
                [128, ntw, 128])
            io3 = iota[:, None, :128].to_broadcast([128, ntw, 128])
            nc.vector.tensor_tensor(
                out=oh[:, :ntw * 128].rearrange("p (t j) -> p t j", j=128),
                in0=dr3, in1=io3, op=OP.is_equal)
            return oh

        def gather_group(g, table):
            gb = gb_pool.tile([128, MAXGC * 64], F32, tag="gb")
            if no_gather:
                nc.vector.memset(gb[:], 0.0)
                return gb, None
            nA, nB = g["gA"] * 128, g["gB"] * 128
            calls = []
            if nA:
                calls.append(nc.gpsimd.dma_gather(
                    out_ap=gb[:, :g["gA"] * 64].rearrange(
                        "p (t f) -> p t f", f=64),
                    in_ap=table.ap()[:HALF, :],
                    idxs_ap=ixa[:, g["colA"]:g["colA"] + nA // 16],
                    num_idxs=nA, num_idxs_reg=nA, elem_size=64,
                    single_packet=False))
            if nB:
                calls.append(nc.gpsimd.dma_gather(
                    out_ap=gb[:, g["gA"] * 64:(g["gA"] + g["gB"]) * 64
                              ].rearrange("p (t f) -> p t f", f=64),
                    in_ap=table.ap()[HALF:, :],
                    idxs_ap=ixb[:, g["colB"]:g["colB"] + nB // 16],
                    num_idxs=nB, num_idxs_reg=nB, elem_size=64,
                    single_packet=False))
            fence_t = fn_pool.tile([128, 64], F32, tag="fence")
            fence = nc.gpsimd.dma_gather(
                out_ap=fence_t[:].rearrange("p (t f) -> p t f", f=64),
                in_ap=table.ap()[:HALF, :],
                idxs_ap=fence_ix[:],
                num_idxs=128, num_idxs_reg=128, elem_size=64,
                single_packet=True)
            for call in calls:
                add_dep_helper(fence.ins, call.ins, True, "fence>call")
            return gb, fence

        def gb_col(g, wl, t):
            """gb chunk index for window-in-group wl, window tile t."""
            ntAw = int(g["ntA"][wl])
            if t < ntAw:
                return int(g["cA0"][wl]) + t
            return g["gA"] + int(g["cB0"][wl]) + (t - ntAw)

        def scatter_win(oh, lhs_cols, fence, F, self_lhs=None):
            """lhs_cols: (buf, col) per gather tile; self_lhs: (buf, col)
            whose rhs is the identity (adds own-node values)."""
            ps = psS.tile([64, 128], F32, space="PSUM", tag="psS")
            if no_mm:
                nc.vector.memset(ps[:], 0.0)
                return ps
            n = len(lhs_cols) + (1 if self_lhs is not None else 0)
            for t, (buf, col) in enumerate(lhs_cols):
                mm = nc.tensor.matmul(
                    out=ps[:F, :], lhsT=buf[:, col:col + F],
                    rhs=oh[:, t * 128:(t + 1) * 128],
                    start=(t == 0), stop=(t == n - 1))
                if fence is not None:
                    add_dep_helper(mm.ins, fence.ins, True, "mm>fence")
            if self_lhs is not None:
                buf, col = self_lhs
                nc.tensor.matmul(
                    out=ps[:F, :], lhsT=buf[:, col:col + F],
                    rhs=ident[:128, :128], start=(len(lhs_cols) == 0),
                    stop=True)
            return ps

        for _rep in range(reps):
            # ---- Layer 1 (no gather) ---------------------------------------
            for w in range(NW):
                oh = onehot_win(w)
                cols = [(ys, int(plan.tbase[w]) + t)
                        for t in range(int(plan.nt[w]))]
                ps1 = scatter_win(oh, cols, None, 1, self_lhs=(xd_nm, w))
                a = ev1.tile([1, 128], F32, tag="a")
                nc.vector.tensor_tensor(out=a[:], in0=ps1[:1, :],
                                        in1=dinvT[:1, wsl(w)], op=OP.mult)
                sp = ev2.tile([1, 128], F32, tag="sp")
                nc.scalar.activation(out=sp[:], in_=a[:], func=AF.Relu)
                sm = ev2.tile([1, 128], F32, tag="sm")
                nc.scalar.activation(out=sm[:], in_=a[:], func=AF.Relu,
                                     scale=-1.0)
                spd = ev3.tile([1, 128], F32, tag="spd")
                nc.vector.tensor_tensor(out=spd[:], in0=sp[:],
                                        in1=dinvT[:1, wsl(w)], op=OP.mult)
                smd = ev3.tile([1, 128], F32, tag="smd")
                nc.vector.tensor_tensor(out=smd[:], in0=sm[:],
                                        in1=dinvT[:1, wsl(w)], op=OP.mult)
                psN = psT.tile([128, 128], F32, space="PSUM", tag="psN")
                nc.tensor.transpose(out=psN[:, 0:1], in_=spd[:],
                                    identity=ident[:1, :1])
                nc.tensor.transpose(out=psN[:, 1:2], in_=smd[:],
                                    identity=ident[:1, :1])
                nc.scalar.activation(out=staging2[:, w * 2:(w + 1) * 2],
                                     in_=psN[:, :2], func=AF.Copy)

            nc.sync.dma_start(
                out=t2c_sl.ap()[:].rearrange("(w p) f -> p w f", p=128),
                in_=staging2[:].rearrange("p (w f) -> p w f", f=2))
            if no_coll:
                nc.gpsimd.dma_start(out=t2c_full.ap()[:NWS, :],
                                    in_=t2c_sl.ap()[:])
            else:
                nc.gpsimd.collective_compute(
                    "AllGather", OP.bypass, replica_groups=rg,
                    ins=[t2c_sl.ap()[:]], outs=[t2c_full.ap()[:]])
            # expand compact [N,2] into full 256B rows (s+,s- repeated) via
            # SBUF bounce -- avoids a 50k-descriptor strided DRAM write
            t2sb = stg.tile([128, 2 * PAD_N // 128], F32, tag="t2sb")
            nc.sync.dma_start(
                out=t2sb[:].rearrange("p (w f) -> p w f", f=2),
                in_=t2c_full.ap()[:].rearrange("(w p) f -> p w f", p=128))
            for ch in range(8):
                exp = expp.tile([128, NW * 64], F32, tag="exp")
                seg = t2sb[:, ch * 2 * NW:(ch + 1) * 2 * NW].rearrange(
                    "p (w f) -> p w f", f=2)
                nc.vector.tensor_copy(
                    out=exp[:].rearrange("p (w d f) -> p w d f", d=32, f=2),
                    in_=seg[:, :, None, :].to_broadcast([128, NW, 32, 2]))
                nc.sync.dma_start(
                    out=t2.ap()[ch * NWS:(ch + 1) * NWS, :].rearrange(
                        "(w p) f -> p w f", p=128),
                    in_=exp[:].rearrange("p (w f) -> p w f", f=64))

            # ---- Layer 2 ---------------------------------------------------
            for g in plan.groups:
                gb, fence = gather_group(g, t2)
                for wl in range(g["gn"]):
                    w = g["ws"] + wl
                    oh = onehot_win(w)
                    cols = [(gb, gb_col(g, wl, t) * 64)
                            for t in range(int(plan.nt[w]))]
                    ps2 = scatter_win(oh, cols, fence, 2,
                                      self_lhs=(staging2, w * 2))
                    pq = ev1.tile([2, 128], F32, tag="pq")
                    nc.vector.tensor_tensor(out=pq[:], in0=ps2[:2, :],
                                            in1=dinvT[:2, wsl(w)],
                                            op=OP.mult)
                    if t3local:
                        # p~,q~ = dinv * (p,q); table row built locally later
                        nc.vector.tensor_tensor(out=staging2c[:, wsl(w)],
                                                in0=pq[:],
                                                in1=dinvT[:2, wsl(w)],
                                                op=OP.mult)
                    psh = psZ.tile([128, 256], F32, space="PSUM", tag="psz")
                    nc.tensor.matmul(out=psh[:, :128], lhsT=UVT[:], rhs=pq[:],
                                     start=True, stop=True)
                    h2 = ev2.tile([128, 128], F32, tag="h2")
                    nc.scalar.activation(out=h2[:], in_=psh[:, :128],
                                         func=AF.Relu, bias=b2[:])
                    pst3 = psZ.tile([128, 256], F32, space="PSUM", tag="psz")
                    nc.tensor.matmul(out=pst3[:64, :128], lhsT=w3[:],
                                     rhs=h2[:], start=True, stop=True)
                    g3 = ev3.tile([64, 128], F32, tag="g3")
                    nc.vector.tensor_tensor(out=g3[:],
                                            in0=pst3[:64, :128],
                                            in1=dinvT[:64, wsl(w)],
                                            op=OP.mult)
                    psN = psT.tile([128, 128], F32, space="PSUM", tag="psN")
                    nc.tensor.transpose(out=psN[:, :64], in_=g3[:],
                                        identity=ident[:64, :64])
                    nc.scalar.activation(out=staging[:, w * 64:(w + 1) * 64],
                                         in_=psN[:, :64], func=AF.Copy)

            if t3local:
                nc.sync.dma_start(out=t3c_sl.ap()[:], in_=staging2c[:])
                if no_coll:
                    nc.gpsimd.dma_start(out=t3c_full.ap()[:2, :],
                                        in_=t3c_sl.ap()[:])
                else:
                    nc.gpsimd.collective_compute(
                        "AllGather", OP.bypass, replica_groups=rg,
                        ins=[t3c_sl.ap()[:]], outs=[t3c_full.ap()[:]])
                # local T3 build: row n = relu(p~ u + q~ v) @ W3  (b2 == 0)
                for c8 in range(C):
                    pqsb = stg.tile([2, NWS], F32, tag="pqsb")
                    nc.sync.dma_start(out=pqsb[:],
                                      in_=t3c_full.ap()[2 * c8:2 * c8 + 2, :])
                    tstag = expp.tile([128, NW * 64], F32, tag="exp")
                    for wc in range(NW):
                        psh = psZ.tile([128, 256], F32, space="PSUM",
                                       tag="psz")
                        nc.tensor.matmul(out=psh[:, :128], lhsT=UVT[:],
                                         rhs=pqsb[:, wc * 128:(wc + 1) * 128],
                                         start=True, stop=True)
                        h2b = ev2.tile([128, 128], F32, tag="h2")
                        nc.scalar.activation(out=h2b[:], in_=psh[:, :128],
                                             func=AF.Relu)
                        pst = psZ.tile([128, 256], F32, space="PSUM",
                                       tag="psz")
                        nc.tensor.matmul(out=pst[:64, :128], lhsT=w3[:],
                                         rhs=h2b[:], start=True, stop=True)
                        t3v = ev3.tile([64, 128], F32, tag="g3")
                        nc.scalar.activation(out=t3v[:], in_=pst[:64, :128],
                                             func=AF.Copy)
                        psN = psT.tile([128, 128], F32, space="PSUM",
                                       tag="psN")
                        nc.tensor.transpose(out=psN[:, :64], in_=t3v[:],
                                            identity=ident[:64, :64])
                        nc.scalar.activation(
                            out=tstag[:, wc * 64:(wc + 1) * 64],
                            in_=psN[:, :64], func=AF.Copy)
                    nc.sync.dma_start(
                        out=t3.ap()[c8 * NWS:(c8 + 1) * NWS, :].rearrange(
                            "(w p) f -> p w f", p=128),
                        in_=tstag[:].rearrange("p (w f) -> p w f", f=64))
            else:
                nc.sync.dma_start(
                    out=y3_sl.ap()[:].rearrange("(w p) f -> p w f", p=128),
                    in_=staging[:].rearrange("p (w f) -> p w f", f=64))
                if no_coll:
                    nc.gpsimd.dma_start(out=y3_full.ap()[:NWS, :],
                                        in_=y3_sl.ap()[:])
                else:
                    nc.gpsimd.collective_compute(
                        "AllGather", OP.bypass, replica_groups=rg,
                        ins=[y3_sl.ap()[:]], outs=[y3_full.ap()[:]])

            # ---- Layer 3 + pooling ----------------------------------------
            pooled_a = psHold.tile([128, 64], F32, space="PSUM", tag="pool_a")
            pooled_b = psHold.tile([128, 64], F32, space="PSUM", tag="pool_b")
            for g in plan.groups:
                gb, fence = gather_group(g, t3 if t3local else y3_full)
                for wl in range(g["gn"]):
                    w = g["ws"] + wl
                    oh = onehot_win(w)
                    cols = [(gb, gb_col(g, wl, t) * 64)
                            for t in range(int(plan.nt[w]))]
                    ps3 = scatter_win(oh, cols, fence, 64,
                                      self_lhs=(staging, w * 64))
                    agg = ev1.tile([64, 128], F32, tag="agg")
                    nc.vector.tensor_tensor(out=agg[:], in0=ps3[:64, :],
                                            in1=dinvT[:64, wsl(w)],
                                            op=OP.mult)
                    h3 = ev2.tile([64, 128], F32, tag="h3")
                    nc.scalar.activation(out=h3[:], in_=agg[:], func=AF.Relu,
                                         bias=b3[:])
                    psN = psT.tile([128, 128], F32, space="PSUM", tag="psN")
                    nc.tensor.transpose(out=psN[:, :64], in_=h3[:],
                                        identity=ident[:64, :64])
                    h3nm = ev3.tile([128, 64], F32, tag="h3nm")
                    nc.scalar.activation(out=h3nm[:], in_=psN[:, :64],
                                         func=AF.Copy)
                    ohp = oh_pool.tile([128, GP], F32, tag="ohp")
                    bc = nd_batch[:, w:w + 1].to_broadcast([128, GP])
                    nc.vector.tensor_tensor(out=ohp[:], in0=bc, in1=iota[:],
                                            op=OP.is_equal)
                    for half, ps_pool in ((0, pooled_a), (1, pooled_b)):
                        lhs = ohp[:, half * 128:(half + 1) * 128]
                        nc.tensor.matmul(out=ps_pool[:], lhsT=lhs,
                                         rhs=h3nm[:],
                                         start=(w == 0), stop=(w == NW - 1))

            # ---- finale ----------------------------------------------------
            pa = setup.tile([128, 64], F32, tag="pa")
            pb = setup.tile([128, 64], F32, tag="pb")
            nc.scalar.activation(out=pa[:], in_=pooled_a[:], func=AF.Copy)
            nc.scalar.activation(out=pb[:], in_=pooled_b[:], func=AF.Copy)
            nc.sync.dma_start(out=pool_in.ap()[0:128, :], in_=pa[:])
            nc.sync.dma_start(out=pool_in.ap()[128:256, :], in_=pb[:])
            if no_coll:
                nc.gpsimd.dma_start(out=pool_out.ap()[:],
                                    in_=pool_in.ap()[:])
            else:
                nc.gpsimd.collective_compute(
                    "AllReduce", OP.add, replica_groups=rg,
                    ins=[pool_in.ap()[:]], outs=[pool_out.ap()[:]])

        meanT = setup.tile([64, 256], F32, tag="meanT")
        for half in (0, 1):
            pl = setup.tile([128, 64], F32, tag=f"pl{half}")
            nc.sync.dma_start(
                out=pl[:], in_=pool_out.ap()[half * 128:(half + 1) * 128, :])
            cntm = setup.tile([128, 1], F32, tag=f"cntm{half}")
            nc.vector.tensor_scalar_max(out=cntm[:],
                                        in0=cnts[:, half:half + 1],
                                        scalar1=1.0)
            rc = setup.tile([128, 1], F32, tag=f"rc{half}")
            nc.vector.reciprocal(out=rc[:], in_=cntm[:])
            mean = setup.tile([128, 64], F32, tag=f"mean{half}")
            nc.vector.tensor_scalar_mul(out=mean[:], in0=pl[:],
                                        scalar1=rc[:])
            psMT = psT.tile([128, 128], F32, space="PSUM", tag="psN")
            nc.tensor.transpose(out=psMT[:64, :], in_=mean[:],
                                identity=ident[:])
            nc.scalar.activation(out=meanT[:, half * 128:(half + 1) * 128],
                                 in_=psMT[:64, :], func=AF.Copy)

        psZ1 = psZ.tile([128, 256], F32, space="PSUM", tag="psz")
        nc.tensor.matmul(out=psZ1[:32, :256], lhsT=l1w[:], rhs=meanT[:],
                         start=True, stop=True)
        z1 = setup.tile([32, 256], F32, tag="z1")
        nc.scalar.activation(out=z1[:], in_=psZ1[:32, :256], func=AF.Relu,
                             bias=l1b[:])
        for half in (0, 1):
            psO = psT.tile([128, 128], F32, space="PSUM", tag="psN")
            nc.tensor.matmul(out=psO[:, :1],
                             lhsT=z1[:, half * 128:(half + 1) * 128],
                             rhs=l2w[:], start=True, stop=True)
            ob = setup.tile([128, 1], F32, tag=f"ob{half}")
            nc.scalar.activation(out=ob[:], in_=psO[:, :1],
                                 func=AF.Identity, bias=l2b[:])
            nc.sync.dma_start(out=out_d.ap()[half * 128:(half + 1) * 128, :],
                              in_=ob[:])

    nc.compile()
    return nc


# ----------------------------------------------------------------------------
# Runner
# ----------------------------------------------------------------------------

_CACHE = {}


def get_program(plan, reps=1, **kw):
    key = plan.key() + (reps,) + tuple(sorted(kw.items()))
    if key not in _CACHE:
        _CACHE[key] = build_program(plan, reps, **kw)
    return _CACHE[key]


def run(plan, inputs, trace=False):
    in_maps = prep_inputs(plan, **inputs)
    nc = get_program(plan)
    res = bass_utils.run_bass_kernel_spmd(
        nc, in_maps, core_ids=list(range(C)), trace=trace)
    out = res.results[0]["out"][:N_GRAPHS, :].astype(np.float32)
    return out, res


def kernel(**inputs) -> np.ndarray:
    plan = Plan(inputs["edge_index"])
    out, _ = run(plan, inputs)
    return out


# revision 13
# speedup vs baseline: 52.9826x; 1.8002x over previous
"""GCN binding-affinity model on 8 TRN2 cores — v2.

Structural changes vs v1 (kernel.py):
  * L1 aggregation needs NO device gather: per-edge x[src] / deg[src] are
    shipped as host-sharded edge data (input sharding), normalized and
    scatter-summed on device (F=1 one-hot matmuls).
  * h1 = relu(outer(a, w1)) is exactly rank 2 (b1 == 0 per problem spec):
    h1 = relu(a) w1+ + relu(-a) w1-.  So L2's message table is just the two
    scalars s+/- = dinv * relu(+-a) per node -> AllGather 400KB instead of
    12.8MB; the 64-wide expansion happens AFTER aggregation via the fixed
    vectors u = relu(w1)@W2, v = relu(-w1)@W2.
  * Self-loop contributions are applied locally (not as gathered edges).
  * Exact per-window tile packing (variable ntA/ntB per window) instead of
    global caps -> ~20% fewer gather descriptors / matmuls / one-hots.
  * Gather tables are [*, 64] f32 rows (256B, the SWDGE minimum); for the
    scalar L2 table only cols 0:2 hold data (lhsT never reads the rest).

Math identical to reference:
  per layer: agg = dinv (.) ((A+I) (dinv (.) t)), dinv = rsqrt(indeg+1).
  L1 t = x (scalar), L2 t = h1 (rank 2 -> 2 scalar channels), L3 t = h2@W3.
"""

import os
import sys
from contextlib import ExitStack

import numpy as np

for _p in ("/opt/trn_rl_repo",):
    if _p not in sys.path and os.path.isdir(_p):
        sys.path.insert(0, _p)

import concourse.bass as bass
import concourse.mybir as mybir
import concourse.tile as tile
from concourse import bacc
from concourse import bass_utils
from concourse.masks import make_identity
from concourse.tile_rust import add_dep_helper

F32 = mybir.dt.float32
I16 = mybir.dt.int16
AF = mybir.ActivationFunctionType
OP = mybir.AluOpType

N_NODES = 50000
N_EDGES = 600000
N_GRAPHS = 256
C = 8
NW = 49
NWS = NW * 128          # 6272 nodes per core
PAD_N = C * NWS         # 50176
HALF = 32768
GP = 256
GSZ = 4                 # windows per gather group


def wrap16(vals):
    return np.ascontiguousarray(vals.reshape(-1, 16).T.astype(np.int16))


def rep8(block):
    return np.tile(block, (8, 1))


class Plan:
    """Per-core edge packing metadata (topology-derived, shared across cores
    as parallel lists)."""

    def __init__(self, edge_index):
        src = np.asarray(edge_index[0], dtype=np.int64)
        dst = np.asarray(edge_index[1], dtype=np.int64)
        self.deg = np.bincount(dst, minlength=N_NODES).astype(np.float32)

        core_of = dst // NWS
        order0 = np.argsort(core_of, kind="stable")
        src, dst, core_sorted = src[order0], dst[order0], core_of[order0]

        self.cores = []
        for c in range(C):
            base = c * NWS
            lo = np.searchsorted(core_sorted, c, side="left")
            hi = np.searchsorted(core_sorted, c, side="right")
            c_src, c_dst = src[lo:hi], dst[lo:hi]
            w_of = (c_dst - base) >> 7
            is_a = c_src < HALF
            key = w_of * 2 + (~is_a)
            order = np.argsort(key, kind="stable")
            c_src, c_dst, w_of, is_a, key = (
                c_src[order], c_dst[order], w_of[order], is_a[order],
                key[order])
            kstart = np.searchsorted(key, np.arange(2 * NW), side="left")
            kend = np.searchsorted(key, np.arange(2 * NW), side="right")
            cnt = kend - kstart
            cntA, cntB = cnt[0::2], cnt[1::2]
            ntA = np.ceil(cntA / 128).astype(int)
            ntB = np.ceil(cntB / 128).astype(int)
            nt = ntA + ntB
            rank = np.arange(len(key)) - kstart[key]  # rank within (w, half)

            tbase = np.concatenate([[0], np.cumsum(nt)]).astype(int)
            T_total = int(tbase[-1])

            # groups
            groups = []
            offA = [0]
            offB = [0]
            for ws in range(0, NW, GSZ):
                gn = min(GSZ, NW - ws)
                wl = np.arange(ws, ws + gn)
                gA = int(ntA[wl].sum())
                gB = int(ntB[wl].sum())
                cA0 = np.concatenate([[0], np.cumsum(ntA[wl])]).astype(int)
                cB0 = np.concatenate([[0], np.cumsum(ntB[wl])]).astype(int)
                groups.append(dict(ws=ws, gn=gn, gA=gA, gB=gB,
                                   cA0=cA0, cB0=cB0,
                                   ntA=ntA[wl].copy(), ntB=ntB[wl].copy(),
                                   colA=offA[-1], colB=offB[-1]))
                offA.append(offA[-1] + gA * 128 // 16)
                offB.append(offB[-1] + gB * 128 // 16)
            TOT_A, TOT_B = offA[-1], offB[-1]

            # per-edge global tile index + lane
            gt = np.where(
                is_a,
                tbase[w_of] + rank // 128,
                tbase[w_of] + ntA[w_of] + rank // 128)
            lane = rank % 128

            # per-edge position within the group idx sequences
            gi_of = w_of // GSZ
            grpA_off = np.zeros(NW, int)
            grpB_off = np.zeros(NW, int)
            for g in groups:
                ws, gn = g["ws"], g["gn"]
                grpA_off[ws:ws + gn] = g["cA0"][:gn] * 128
                grpB_off[ws:ws + gn] = g["cB0"][:gn] * 128
            colA_base = np.array([g["colA"] * 16 for g in groups])
            colB_base = np.array([g["colB"] * 16 for g in groups])
            posA = colA_base[gi_of] + grpA_off[w_of] + rank
            posB = colB_base[gi_of] + grpB_off[w_of] + rank

            seqA = np.zeros(TOT_A * 16, np.int64)
            seqB = np.zeros(TOT_B * 16, np.int64)
            seqA[posA[is_a]] = c_src[is_a]
            seqB[posB[~is_a]] = c_src[~is_a] - HALF

            self.cores.append(dict(
                base=base, src=c_src, dst=c_dst, w_of=w_of, is_a=is_a,
                rank=rank, ntA=ntA, ntB=ntB, nt=nt, tbase=tbase,
                T_total=T_total, groups=groups, TOT_A=TOT_A, TOT_B=TOT_B,
                gt=gt, lane=lane, seqA=seqA, seqB=seqB))

        self.maxT = max(cc["T_total"] for cc in self.cores)
        self.maxNT = max(int(cc["nt"].max()) for cc in self.cores)
        self.maxTOT_A = max(cc["TOT_A"] for cc in self.cores)
        self.maxTOT_B = max(cc["TOT_B"] for cc in self.cores)
        self.max_gC = max(g["gA"] + g["gB"]
                          for cc in self.cores for g in cc["groups"])
        # the device program is built once from core 0's plan shape; all
        # cores must share the same unrolled structure -> pad counts to the
        # max across cores.  Instead of that complexity: build per-core
        # programs?  SPMD needs ONE program.  So we equalize the plan:
        self._equalize()

    def _equalize(self):
        """Pad every core's per-window tile counts up to the max across
        cores so a single SPMD program fits all cores."""
        ntA_max = np.max([cc["ntA"] for cc in self.cores], axis=0)
        ntB_max = np.max([cc["ntB"] for cc in self.cores], axis=0)
        nt = ntA_max + ntB_max
        tbase = np.concatenate([[0], np.cumsum(nt)]).astype(int)
        T_total = int(tbase[-1])
        groups = []
        offA = [0]
        offB = [0]
        for ws in range(0, NW, GSZ):
            gn = min(GSZ, NW - ws)
            wl = np.arange(ws, ws + gn)
            gA = int(ntA_max[wl].sum())
            gB = int(ntB_max[wl].sum())
            cA0 = np.concatenate([[0], np.cumsum(ntA_max[wl])]).astype(int)
            cB0 = np.concatenate([[0], np.cumsum(ntB_max[wl])]).astype(int)
            groups.append(dict(ws=ws, gn=gn, gA=gA, gB=gB, cA0=cA0, cB0=cB0,
                               ntA=ntA_max[wl].copy(), ntB=ntB_max[wl].copy(),
                               colA=offA[-1], colB=offB[-1]))
            offA.append(offA[-1] + gA * 128 // 16)
            offB.append(offB[-1] + gB * 128 // 16)
        TOT_A, TOT_B = offA[-1], offB[-1]

        grpA_off = np.zeros(NW, int)
        grpB_off = np.zeros(NW, int)
        colA_base = np.zeros(NW, int)
        colB_base = np.zeros(NW, int)
        for g in groups:
            ws, gn = g["ws"], g["gn"]
            grpA_off[ws:ws + gn] = g["cA0"][:gn] * 128
            grpB_off[ws:ws + gn] = g["cB0"][:gn] * 128
            colA_base[ws:ws + gn] = g["colA"] * 16
            colB_base[ws:ws + gn] = g["colB"] * 16

        for cc in self.cores:
            w_of, is_a, rank = cc["w_of"], cc["is_a"], cc["rank"]
            gt = np.where(is_a,
                          tbase[w_of] + rank // 128,
                          tbase[w_of] + ntA_max[w_of] + rank // 128)
            posA = colA_base[w_of] + grpA_off[w_of] + rank
            posB = colB_base[w_of] + grpB_off[w_of] + rank
            seqA = np.zeros(TOT_A * 16, np.int64)
            seqB = np.zeros(TOT_B * 16, np.int64)
            seqA[posA[is_a]] = cc["src"][is_a]
            seqB[posB[~is_a]] = cc["src"][~is_a] - HALF
            cc.update(gt=gt, seqA=seqA, seqB=seqB)

        self.ntA = ntA_max
        self.ntB = ntB_max
        self.nt = nt
        self.tbase = tbase
        self.T_total = T_total
        self.groups = groups
        self.TOT_A = TOT_A
        self.TOT_B = TOT_B
        self.max_gC = max(g["gA"] + g["gB"] for g in groups)

    def key(self):
        return (self.T_total, self.TOT_A, self.TOT_B,
                tuple(self.nt.tolist()))


def prep_inputs(plan, x, W1, b1, W2, b2, W3, b3, lin1_w, lin1_b, lin2_w,
                lin2_b, edge_index, batch):
    assert not np.any(np.asarray(b1)), "v2 exploits b1 == 0 (rank-2 h1)"
    assert not np.any(np.asarray(b2)), "t3local exploits b2 == 0"
    x = np.asarray(x, np.float32).reshape(-1)
    batch = np.asarray(batch, dtype=np.int64)
    deg = plan.deg

    x_ext = np.zeros(PAD_N, np.float32); x_ext[:N_NODES] = x
    deg_ext = np.zeros(PAD_N, np.float32); deg_ext[:N_NODES] = deg
    batch_ext = np.full(PAD_N, -1.0, np.float32)
    batch_ext[:N_NODES] = batch.astype(np.float32)

    iota = np.broadcast_to(np.arange(GP, dtype=np.float32),
                           (128, GP)).copy()
    cnts = np.bincount(batch, minlength=GP).astype(np.float32)
    cnts2 = np.ascontiguousarray(cnts.reshape(2, 128).T)

    in_maps = []
    for c, cc in enumerate(plan.cores):
        T = plan.T_total
        drel = np.full((128, T), -1.0, np.float32)
        ysx = np.zeros((128, T), np.float32)
        ysd = np.zeros((128, T), np.float32)
        gt, lane = cc["gt"], cc["lane"]
        drel[lane, gt] = (cc["dst"] - cc["base"] - (cc["w_of"] << 7)
                          ).astype(np.float32)
        ysx[lane, gt] = x[cc["src"]]
        ysd[lane, gt] = deg[cc["src"]]

        ixa = rep8(wrap16(cc["seqA"]))
        ixb = rep8(wrap16(cc["seqB"]))

        base = cc["base"]
        sl = slice(base, base + NWS)
        nd = lambda a: np.ascontiguousarray(a[sl].reshape(NW, 128).T)
        degT = np.broadcast_to(deg_ext[sl][None, :], (128, NWS)).copy()

        in_maps.append({
            "ixa": ixa, "ixb": ixb,
            "drel": drel, "ysx": ysx, "ysd": ysd,
            "nd_batch": nd(batch_ext),
            "nd_x": nd(x_ext), "nd_deg": nd(deg_ext),
            "degT": degT,
            "iota": iota, "cnts": cnts2,
            "w1": np.asarray(W1, np.float32).reshape(1, 64),
            "w2": np.asarray(W2, np.float32).reshape(64, 128),
            "w3": np.asarray(W3, np.float32).reshape(128, 64),
            "b2": np.asarray(b2, np.float32).reshape(128, 1),
            "b3": np.asarray(b3, np.float32).reshape(64, 1),
            "l1w": np.asarray(lin1_w, np.float32).reshape(64, 32),
            "l1b": np.asarray(lin1_b, np.float32).reshape(32, 1),
            "l2w": np.asarray(lin2_w, np.float32).reshape(32, 1),
            "l2b": np.full((128, 1),
                           np.float32(np.asarray(lin2_b).reshape(())),
                           np.float32),
        })
    return in_maps


# ----------------------------------------------------------------------------
# Device program
# ----------------------------------------------------------------------------

def build_program(plan, reps=1, no_coll=False, no_gather=False,
                  no_mm=False, no_oh=False, t3local=False):
    rg = [list(range(C))]
    T = plan.T_total
    TOT_A, TOT_B = plan.TOT_A, plan.TOT_B
    MAXNT = int(plan.nt.max())
    MAXGC = plan.max_gC

    nc = bacc.Bacc("TRN2", target_bir_lowering=False, debug=False,
                   num_devices=C, num_swdge_queues=2)

    din = {}
    def inp(name, shape, dt=F32):
        din[name] = nc.dram_tensor(name, list(shape), dt,
                                   kind="ExternalInput")
        return din[name]

    inp("ixa", (128, TOT_A), I16)
    inp("ixb", (128, TOT_B), I16)
    inp("drel", (128, T)); inp("ysx", (128, T)); inp("ysd", (128, T))
    inp("nd_batch", (128, NW))
    inp("nd_x", (128, NW)); inp("nd_deg", (128, NW))
    inp("degT", (128, NWS))
    inp("iota", (128, GP)); inp("cnts", (128, 2))
    inp("w1", (1, 64)); inp("w2", (64, 128)); inp("w3", (128, 64))
    inp("b2", (128, 1)); inp("b3", (64, 1))
    inp("l1w", (64, 32)); inp("l1b", (32, 1)); inp("l2w", (32, 1))
    inp("l2b", (128, 1))

    out_d = nc.dram_tensor("out", [GP, 1], F32, kind="ExternalOutput")

    t2c_sl = nc.dram_tensor("t2c_slice", [NWS, 2], F32, kind="Internal")
    t2c_full = nc.dram_tensor("t2c_full", [PAD_N, 2], F32, kind="Internal",
                              addr_space="Shared")
    t2 = nc.dram_tensor("t2", [PAD_N, 64], F32, kind="Internal")
    y3_sl = nc.dram_tensor("y3_slice", [NWS, 64], F32, kind="Internal")
    y3_full = nc.dram_tensor("y3_full", [PAD_N, 64], F32, kind="Internal",
                             addr_space="Shared")
    t3c_sl = nc.dram_tensor("t3c_slice", [2, NWS], F32, kind="Internal")
    t3c_full = nc.dram_tensor("t3c_full", [2 * C, NWS], F32, kind="Internal",
                              addr_space="Shared")
    t3 = nc.dram_tensor("t3", [PAD_N, 64], F32, kind="Internal")
    pool_in = nc.dram_tensor("pool_in", [GP, 64], F32, kind="Internal")
    pool_out = nc.dram_tensor("pool_out", [GP, 64], F32, kind="Internal",
                              addr_space="Shared")

    with tile.TileContext(nc) as tc, ExitStack() as ctx:
        P = ctx.enter_context
        setup = P(tc.tile_pool(name="setup", bufs=1))
        oh_pool = P(tc.tile_pool(name="oh", bufs=2))
        gb_pool = P(tc.tile_pool(name="gb", bufs=2))
        fn_pool = P(tc.tile_pool(name="fn", bufs=2))
        psS = P(tc.tile_pool(name="psS", bufs=2, space="PSUM"))
        psZ = P(tc.tile_pool(name="psZ", bufs=2, space="PSUM"))
        psT = P(tc.tile_pool(name="psT", bufs=2, space="PSUM"))
        psHold = P(tc.tile_pool(name="psHold", bufs=1, space="PSUM"))
        ev1 = P(tc.tile_pool(name="ev1", bufs=3))
        ev2 = P(tc.tile_pool(name="ev2", bufs=3))
        ev3 = P(tc.tile_pool(name="ev3", bufs=3))
        stg = P(tc.tile_pool(name="stg", bufs=1))
        expp = P(tc.tile_pool(name="expp", bufs=2))

        def load(name, shape, dt=F32):
            t = setup.tile(list(shape), dt, tag=name)
            nc.sync.dma_start(out=t[:], in_=din[name].ap()[:])
            return t

        ixa = load("ixa", (128, TOT_A), I16)
        ixb = load("ixb", (128, TOT_B), I16)
        drel = load("drel", (128, T))
        ysx = load("ysx", (128, T))
        ysd = load("ysd", (128, T))
        nd_batch = load("nd_batch", (128, NW))
        nd_x = load("nd_x", (128, NW))
        nd_deg = load("nd_deg", (128, NW))
        degT = load("degT", (128, NWS))
        iota = load("iota", (128, GP))
        cnts = load("cnts", (128, 2))
        w1 = load("w1", (1, 64)); w2 = load("w2", (64, 128))
        w3 = load("w3", (128, 64))
        b2 = load("b2", (128, 1)); b3 = load("b3", (64, 1))
        l1w = load("l1w", (64, 32)); l1b = load("l1b", (32, 1))
        l2w = load("l2w", (32, 1)); l2b = load("l2b", (128, 1))

        ident = setup.tile([128, 128], F32, tag="ident")
        make_identity(nc, ident[:])
        fence_ix = setup.tile([128, 8], I16, tag="fence_ix")
        nc.vector.memset(fence_ix[:], 0)

        # dinvT = rsqrt(degT + 1) computed in place (free dim layout)
        dinvT = degT
        nc.scalar.activation(out=dinvT[:], in_=degT[:], func=AF.Sqrt,
                             bias=1.0, scale=1.0)
        nc.vector.reciprocal(out=dinvT[:], in_=dinvT[:])

        # per-slot src normalization: ys = x[src] * rsqrt(deg[src]+1)
        nc.scalar.activation(out=ysd[:], in_=ysd[:], func=AF.Sqrt,
                             bias=1.0, scale=1.0)
        nc.vector.reciprocal(out=ysd[:], in_=ysd[:])
        ys = ysx
        nc.vector.tensor_tensor(out=ys[:], in0=ysd[:], in1=ysx[:],
                                op=OP.mult)

        # own-node x*dinv, node-partition layout (L1 self-loop lhsT)
        dinv_nm = setup.tile([128, NW], F32, tag="dinv_nm")
        nc.scalar.activation(out=dinv_nm[:], in_=nd_deg[:], func=AF.Sqrt,
                             bias=1.0, scale=1.0)
        nc.vector.reciprocal(out=dinv_nm[:], in_=dinv_nm[:])
        xd_nm = setup.tile([128, NW], F32, tag="xd_nm")
        nc.vector.tensor_tensor(out=xd_nm[:], in0=nd_x[:], in1=dinv_nm[:],
                                op=OP.mult)

        # UVT = [u; v] = [relu(w1); relu(-w1)] @ W2  -> [2, 128]
        w1p = setup.tile([1, 64], F32, tag="w1p")
        w1m = setup.tile([1, 64], F32, tag="w1m")
        nc.scalar.activation(out=w1p[:], in_=w1[:], func=AF.Relu)
        nc.scalar.activation(out=w1m[:], in_=w1[:], func=AF.Relu,
                             scale=-1.0)
        w1pmT = setup.tile([64, 2], F32, tag="w1pmT")
        for i, src_t in enumerate((w1p, w1m)):
            psx = psT.tile([128, 128], F32, space="PSUM", tag="psN")
            nc.tensor.transpose(out=psx[:64, :1], in_=src_t[:],
                                identity=ident[:1, :1])
            nc.scalar.activation(out=w1pmT[:, i:i + 1], in_=psx[:64, :1],
                                 func=AF.Copy)
        psUV = psZ.tile([128, 256], F32, space="PSUM", tag="psz")
        nc.tensor.matmul(out=psUV[:2, :128], lhsT=w1pmT[:], rhs=w2[:],
                         start=True, stop=True)
        UVT = setup.tile([2, 128], F32, tag="UVT")
        nc.scalar.activation(out=UVT[:], in_=psUV[:2, :128], func=AF.Copy)

        staging = stg.tile([128, NW * 64], F32, tag="staging")
        staging2 = stg.tile([128, NW * 2], F32, tag="staging2")
        if t3local:
            staging2c = stg.tile([2, NWS], F32, tag="staging2c")
        else:
            staging2c = None

        def wsl(w):
            return slice(w * 128, (w + 1) * 128)

        def onehot_win(w):
            ntw = int(plan.nt[w])
            t0 = int(plan.tbase[w])
            oh = oh_pool.tile([128, MAXNT * 128], F32, tag="oh")
            if no_oh or ntw == 0:
                nc.vector.memset(oh[:], 0.0)
                return oh
            dr3 = drel[:, t0:t0 + ntw][:, :, None].to_broadcast(
                [128, ntw, 128])
            io3 = iota[:, None, :128].to_broadcast([128, ntw, 128])
            nc.vector.tensor_tensor(
                out=oh[:, :ntw * 128].rearrange("p (t j) -> p t j", j=128),
                in0=dr3, in1=io3, op=OP.is_equal)
            return oh

        def gather_group(g, table):
            gb = gb_pool.tile([128, MAXGC * 64], F32, tag="gb")
            if no_gather:
                nc.vector.memset(gb[:], 0.0)
                return gb, None
            nA, nB = g["gA"] * 128, g["gB"] * 128
            calls = []
            if nA:
                calls.append(nc.gpsimd.dma_gather(
                    out_ap=gb[:, :g["gA"] * 64].rearrange(
                        "p (t f) -> p t f", f=64),
                    in_ap=table.ap()[:HALF, :],
                    idxs_ap=ixa[:, g["colA"]:g["colA"] + nA // 16],
                    num_idxs=nA, num_idxs_reg=nA, elem_size=64,
                    single_packet=False))
            callsB = []
            if nB:
                callsB.append(nc.gpsimd.dma_gather(
                    out_ap=gb[:, g["gA"] * 64:(g["gA"] + g["gB"]) * 64
                              ].rearrange("p (t f) -> p t f", f=64),
                    in_ap=table.ap()[HALF:, :],
                    idxs_ap=ixb[:, g["colB"]:g["colB"] + nB // 16],
                    num_idxs=nB, num_idxs_reg=nB, elem_size=64,
                    single_packet=False, queue_num=1))
            fence_t = fn_pool.tile([128, 64], F32, tag="fence")
            fence = nc.gpsimd.dma_gather(
                out_ap=fence_t[:].rearrange("p (t f) -> p t f", f=64),
                in_ap=table.ap()[:HALF, :],
                idxs_ap=fence_ix[:],
                num_idxs=128, num_idxs_reg=128, elem_size=64,
                single_packet=True)
            fence_t1 = fn_pool.tile([128, 64], F32, tag="fence1")
            fence1 = nc.gpsimd.dma_gather(
                out_ap=fence_t1[:].rearrange("p (t f) -> p t f", f=64),
                in_ap=table.ap()[:HALF, :],
                idxs_ap=fence_ix[:],
                num_idxs=128, num_idxs_reg=128, elem_size=64,
                single_packet=True, queue_num=1)
            for call in calls:
                add_dep_helper(fence.ins, call.ins, True, "fence>call")
            for call in callsB:
                add_dep_helper(fence1.ins, call.ins, True, "fence1>call")
            add_dep_helper(fence.ins, fence1.ins, True, "fence>fence1")
            return gb, fence

        def gb_col(g, wl, t):
            """gb chunk index for window-in-group wl, window tile t."""
            ntAw = int(g["ntA"][wl])
            if t < ntAw:
                return int(g["cA0"][wl]) + t
            return g["gA"] + int(g["cB0"][wl]) + (t - ntAw)

        def scatter_win(oh, lhs_cols, fence, F, self_lhs=None):
            """lhs_cols: (buf, col) per gather tile; self_lhs: (buf, col)
            whose rhs is the identity (adds own-node values)."""
            ps = psS.tile([64, 128], F32, space="PSUM", tag="psS")
            if no_mm:
                nc.vector.memset(ps[:], 0.0)
                return ps
            n = len(lhs_cols) + (1 if self_lhs is not None else 0)
            for t, (buf, col) in enumerate(lhs_cols):
                mm = nc.tensor.matmul(
                    out=ps[:F, :], lhsT=buf[:, col:col + F],
                    rhs=oh[:, t * 128:(t + 1) * 128],
                    start=(t == 0), stop=(t == n - 1))
                if fence is not None:
                    add_dep_helper(mm.ins, fence.ins, True, "mm>fence")
            if self_lhs is not None:
                buf, col = self_lhs
                nc.tensor.matmul(
                    out=ps[:F, :], lhsT=buf[:, col:col + F],
                    rhs=ident[:128, :128], start=(len(lhs_cols) == 0),
                    stop=True)
            return ps

        for _rep in range(reps):
            # ---- Layer 1 (no gather) ---------------------------------------
            for w in range(NW):
                oh = onehot_win(w)
                cols = [(ys, int(plan.tbase[w]) + t)
                        for t in range(int(plan.nt[w]))]
                ps1 = scatter_win(oh, cols, None, 1, self_lhs=(xd_nm, w))
                a = ev1.tile([1, 128], F32, tag="a")
                nc.vector.tensor_tensor(out=a[:], in0=ps1[:1, :],
                                        in1=dinvT[:1, wsl(w)], op=OP.mult)
                sp = ev2.tile([1, 128], F32, tag="sp")
                nc.scalar.activation(out=sp[:], in_=a[:], func=AF.Relu)
                sm = ev2.tile([1, 128], F32, tag="sm")
                nc.scalar.activation(out=sm[:], in_=a[:], func=AF.Relu,
                                     scale=-1.0)
                spd = ev3.tile([1, 128], F32, tag="spd")
                nc.vector.tensor_tensor(out=spd[:], in0=sp[:],
                                        in1=dinvT[:1, wsl(w)], op=OP.mult)
                smd = ev3.tile([1, 128], F32, tag="smd")
                nc.vector.tensor_tensor(out=smd[:], in0=sm[:],
                                        in1=dinvT[:1, wsl(w)], op=OP.mult)
                psN = psT.tile([128, 128], F32, space="PSUM", tag="psN")
                nc.tensor.transpose(out=psN[:, 0:1], in_=spd[:],
                                    identity=ident[:1, :1])
                nc.tensor.transpose(out=psN[:, 1:2], in_=smd[:],
                                    identity=ident[:1, :1])
                nc.scalar.activation(out=staging2[:, w * 2:(w + 1) * 2],
                                     in_=psN[:, :2], func=AF.Copy)

            nc.sync.dma_start(
                out=t2c_sl.ap()[:].rearrange("(w p) f -> p w f", p=128),
                in_=staging2[:].rearrange("p (w f) -> p w f", f=2))
            if no_coll:
                nc.gpsimd.dma_start(out=t2c_full.ap()[:NWS, :],
                                    in_=t2c_sl.ap()[:])
            else:
                nc.gpsimd.collective_compute(
                    "AllGather", OP.bypass, replica_groups=rg,
                    ins=[t2c_sl.ap()[:]], outs=[t2c_full.ap()[:]])
            # expand compact [N,2] into full 256B rows (s+,s- repeated) via
            # SBUF bounce -- avoids a 50k-descriptor strided DRAM write
            t2sb = stg.tile([128, 2 * PAD_N // 128], F32, tag="t2sb")
            nc.sync.dma_start(
                out=t2sb[:].rearrange("p (w f) -> p w f", f=2),
                in_=t2c_full.ap()[:].rearrange("(w p) f -> p w f", p=128))
            for ch in range(8):
                exp = expp.tile([128, NW * 64], F32, tag="exp")
                seg = t2sb[:, ch * 2 * NW:(ch + 1) * 2 * NW].rearrange(
                    "p (w f) -> p w f", f=2)
                nc.vector.tensor_copy(
                    out=exp[:].rearrange("p (w d f) -> p w d f", d=32, f=2),
                    in_=seg[:, :, None, :].to_broadcast([128, NW, 32, 2]))
                nc.sync.dma_start(
                    out=t2.ap()[ch * NWS:(ch + 1) * NWS, :].rearrange(
                        "(w p) f -> p w f", p=128),
                    in_=exp[:].rearrange("p (w f) -> p w f", f=64))

            # ---- Layer 2 ---------------------------------------------------
            for g in plan.groups:
                gb, fence = gather_group(g, t2)
                for wl in range(g["gn"]):
                    w = g["ws"] + wl
                    oh = onehot_win(w)
                    cols = [(gb, gb_col(g, wl, t) * 64)
                            for t in range(int(plan.nt[w]))]
                    ps2 = scatter_win(oh, cols, fence, 2,
                                      self_lhs=(staging2, w * 2))
                    pq = ev1.tile([2, 128], F32, tag="pq")
                    nc.vector.tensor_tensor(out=pq[:], in0=ps2[:2, :],
                                            in1=dinvT[:2, wsl(w)],
                                            op=OP.mult)
                    if t3local:
                        # p~,q~ = dinv * (p,q); table row built locally later
                        nc.vector.tensor_tensor(out=staging2c[:, wsl(w)],
                                                in0=pq[:],
                                                in1=dinvT[:2, wsl(w)],
                                                op=OP.mult)
                    psh = psZ.tile([128, 256], F32, space="PSUM", tag="psz")
                    nc.tensor.matmul(out=psh[:, :128], lhsT=UVT[:], rhs=pq[:],
                                     start=True, stop=True)
                    h2 = ev2.tile([128, 128], F32, tag="h2")
                    nc.scalar.activation(out=h2[:], in_=psh[:, :128],
                                         func=AF.Relu, bias=b2[:])
                    pst3 = psZ.tile([128, 256], F32, space="PSUM", tag="psz")
                    nc.tensor.matmul(out=pst3[:64, :128], lhsT=w3[:],
                                     rhs=h2[:], start=True, stop=True)
                    g3 = ev3.tile([64, 128], F32, tag="g3")
                    nc.vector.tensor_tensor(out=g3[:],
                                            in0=pst3[:64, :128],
                                            in1=dinvT[:64, wsl(w)],
                                            op=OP.mult)
                    psN = psT.tile([128, 128], F32, space="PSUM", tag="psN")
                    nc.tensor.transpose(out=psN[:, :64], in_=g3[:],
                                        identity=ident[:64, :64])
                    nc.scalar.activation(out=staging[:, w * 64:(w + 1) * 64],
                                         in_=psN[:, :64], func=AF.Copy)

            if t3local:
                nc.sync.dma_start(out=t3c_sl.ap()[:], in_=staging2c[:])
                if no_coll:
                    nc.gpsimd.dma_start(out=t3c_full.ap()[:2, :],
                                        in_=t3c_sl.ap()[:])
                else:
                    nc.gpsimd.collective_compute(
                        "AllGather", OP.bypass, replica_groups=rg,
                        ins=[t3c_sl.ap()[:]], outs=[t3c_full.ap()[:]])
                # local T3 build: row n = relu(p~ u + q~ v) @ W3  (b2 == 0)
                for c8 in range(C):
                    pqsb = stg.tile([2, NWS], F32, tag="pqsb")
                    nc.sync.dma_start(out=pqsb[:],
                                      in_=t3c_full.ap()[2 * c8:2 * c8 + 2, :])
                    tstag = expp.tile([128, NW * 64], F32, tag="exp")
                    for wc in range(NW):
                        psh = psZ.tile([128, 256], F32, space="PSUM",
                                       tag="psz")
                        nc.tensor.matmul(out=psh[:, :128], lhsT=UVT[:],
                                         rhs=pqsb[:, wc * 128:(wc + 1) * 128],
                                         start=True, stop=True)
                        h2b = ev2.tile([128, 128], F32, tag="h2")
                        nc.scalar.activation(out=h2b[:], in_=psh[:, :128],
                                             func=AF.Relu)
                        pst = psZ.tile([128, 256], F32, space="PSUM",
                                       tag="psz")
                        nc.tensor.matmul(out=pst[:64, :128], lhsT=w3[:],
                                         rhs=h2b[:], start=True, stop=True)
                        t3v = ev3.tile([64, 128], F32, tag="g3")
                        nc.scalar.activation(out=t3v[:], in_=pst[:64, :128],
                                             func=AF.Copy)
                        psN = psT.tile([128, 128], F32, space="PSUM",
                                       tag="psN")
                        nc.tensor.transpose(out=psN[:, :64], in_=t3v[:],
                                            identity=ident[:64, :64])
                        nc.scalar.activation(
                            out=tstag[:, wc * 64:(wc + 1) * 64],
                            in_=psN[:, :64], func=AF.Copy)
                    nc.sync.dma_start(
                        out=t3.ap()[c8 * NWS:(c8 + 1) * NWS, :].rearrange(
                            "(w p) f -> p w f", p=128),
                        in_=tstag[:].rearrange("p (w f) -> p w f", f=64))
            else:
                nc.sync.dma_start(
                    out=y3_sl.ap()[:].rearrange("(w p) f -> p w f", p=128),
                    in_=staging[:].rearrange("p (w f) -> p w f", f=64))
                if no_coll:
                    nc.gpsimd.dma_start(out=y3_full.ap()[:NWS, :],
                                        in_=y3_sl.ap()[:])
                else:
                    nc.gpsimd.collective_compute(
                        "AllGather", OP.bypass, replica_groups=rg,
                        ins=[y3_sl.ap()[:]], outs=[y3_full.ap()[:]])

            # ---- Layer 3 + pooling ----------------------------------------
            pooled_a = psHold.tile([128, 64], F32, space="PSUM", tag="pool_a")
            pooled_b = psHold.tile([128, 64], F32, space="PSUM", tag="pool_b")
            for g in plan.groups:
                gb, fence = gather_group(g, t3 if t3local else y3_full)
                for wl in range(g["gn"]):
                    w = g["ws"] + wl
                    oh = onehot_win(w)
                    cols = [(gb, gb_col(g, wl, t) * 64)
                            for t in range(int(plan.nt[w]))]
                    ps3 = scatter_win(oh, cols, fence, 64,
                                      self_lhs=(staging, w * 64))
                    agg = ev1.tile([64, 128], F32, tag="agg")
                    nc.vector.tensor_tensor(out=agg[:], in0=ps3[:64, :],
                                            in1=dinvT[:64, wsl(w)],
                                            op=OP.mult)
                    h3 = ev2.tile([64, 128], F32, tag="h3")
                    nc.scalar.activation(out=h3[:], in_=agg[:], func=AF.Relu,
                                         bias=b3[:])
                    psN = psT.tile([128, 128], F32, space="PSUM", tag="psN")
                    nc.tensor.transpose(out=psN[:, :64], in_=h3[:],
                                        identity=ident[:64, :64])
                    h3nm = ev3.tile([128, 64], F32, tag="h3nm")
                    nc.scalar.activation(out=h3nm[:], in_=psN[:, :64],
                                         func=AF.Copy)
                    ohp = oh_pool.tile([128, GP], F32, tag="ohp")
                    bc = nd_batch[:, w:w + 1].to_broadcast([128, GP])
                    nc.vector.tensor_tensor(out=ohp[:], in0=bc, in1=iota[:],
                                            op=OP.is_equal)
                    for half, ps_pool in ((0, pooled_a), (1, pooled_b)):
                        lhs = ohp[:, half * 128:(half + 1) * 128]
                        nc.tensor.matmul(out=ps_pool[:], lhsT=lhs,
                                         rhs=h3nm[:],
                                         start=(w == 0), stop=(w == NW - 1))

            # ---- finale ----------------------------------------------------
            pa = setup.tile([128, 64], F32, tag="pa")
            pb = setup.tile([128, 64], F32, tag="pb")
            nc.scalar.activation(out=pa[:], in_=pooled_a[:], func=AF.Copy)
            nc.scalar.activation(out=pb[:], in_=pooled_b[:], func=AF.Copy)
            nc.sync.dma_start(out=pool_in.ap()[0:128, :], in_=pa[:])
            nc.sync.dma_start(out=pool_in.ap()[128:256, :], in_=pb[:])
            if no_coll:
                nc.gpsimd.dma_start(out=pool_out.ap()[:],
                                    in_=pool_in.ap()[:])
            else:
                nc.gpsimd.collective_compute(
                    "AllReduce", OP.add, replica_groups=rg,
                    ins=[pool_in.ap()[:]], outs=[pool_out.ap()[:]])

        meanT = setup.tile([64, 256], F32, tag="meanT")
        for half in (0, 1):
            pl = setup.tile([128, 64], F32, tag=f"pl{half}")
            nc.sync.dma_start(
                out=pl[:], in_=pool_out.ap()[half * 128:(half + 1) * 128, :])
            cntm = setup.tile([128, 1], F32, tag=f"cntm{half}")
            nc.vector.tensor_scalar_max(out=cntm[:],
                                        in0=cnts[:, half:half + 1],
                                        scalar1=1.0)
            rc = setup.tile([128, 1], F32, tag=f"rc{half}")
            nc.vector.reciprocal(out=rc[:], in_=cntm[:])
            mean = setup.tile([128, 64], F32, tag=f"mean{half}")
            nc.vector.tensor_scalar_mul(out=mean[:], in0=pl[:],
                                        scalar1=rc[:])
            psMT = psT.tile([128, 128], F32, space="PSUM", tag="psN")
            nc.tensor.transpose(out=psMT[:64, :], in_=mean[:],
                                identity=ident[:])
            nc.scalar.activation(out=meanT[:, half * 128:(half + 1) * 128],
                                 in_=psMT[:64, :], func=AF.Copy)

        psZ1 = psZ.tile([128, 256], F32, space="PSUM", tag="psz")
        nc.tensor.matmul(out=psZ1[:32, :256], lhsT=l1w[:], rhs=meanT[:],
                         start=True, stop=True)
        z1 = setup.tile([32, 256], F32, tag="z1")
        nc.scalar.activation(out=z1[:], in_=psZ1[:32, :256], func=AF.Relu,
                             bias=l1b[:])
        for half in (0, 1):
            psO = psT.tile([128, 128], F32, space="PSUM", tag="psN")
            nc.tensor.matmul(out=psO[:, :1],
                             lhsT=z1[:, half * 128:(half + 1) * 128],
                             rhs=l2w[:], start=True, stop=True)
            ob = setup.tile([128, 1], F32, tag=f"ob{half}")
            nc.scalar.activation(out=ob[:], in_=psO[:, :1],
                                 func=AF.Identity, bias=l2b[:])
            nc.sync.dma_start(out=out_d.ap()[half * 128:(half + 1) * 128, :],
                              in_=ob[:])

    nc.compile()
    return nc


# ----------------------------------------------------------------------------
# Runner
# ----------------------------------------------------------------------------

_CACHE = {}


def get_program(plan, reps=1, **kw):
    key = plan.key() + (reps,) + tuple(sorted(kw.items()))
    if key not in _CACHE:
        _CACHE[key] = build_program(plan, reps, **kw)
    return _CACHE[key]


def run(plan, inputs, trace=False):
    in_maps = prep_inputs(plan, **inputs)
    nc = get_program(plan)
    res = bass_utils.run_bass_kernel_spmd(
        nc, in_maps, core_ids=list(range(C)), trace=trace)
    out = res.results[0]["out"][:N_GRAPHS, :].astype(np.float32)
    return out, res


def kernel(**inputs) -> np.ndarray:
    plan = Plan(inputs["edge_index"])
    out, _ = run(plan, inputs)
    return out


# revision 14
# speedup vs baseline: 53.4741x; 1.0093x over previous
"""GCN binding-affinity model on 8 TRN2 cores — v2.

Structural changes vs v1 (kernel.py):
  * L1 aggregation needs NO device gather: per-edge x[src] / deg[src] are
    shipped as host-sharded edge data (input sharding), normalized and
    scatter-summed on device (F=1 one-hot matmuls).
  * h1 = relu(outer(a, w1)) is exactly rank 2 (b1 == 0 per problem spec):
    h1 = relu(a) w1+ + relu(-a) w1-.  So L2's message table is just the two
    scalars s+/- = dinv * relu(+-a) per node -> AllGather 400KB instead of
    12.8MB; the 64-wide expansion happens AFTER aggregation via the fixed
    vectors u = relu(w1)@W2, v = relu(-w1)@W2.
  * Self-loop contributions are applied locally (not as gathered edges).
  * Exact per-window tile packing (variable ntA/ntB per window) instead of
    global caps -> ~20% fewer gather descriptors / matmuls / one-hots.
  * Gather tables are [*, 64] f32 rows (256B, the SWDGE minimum); for the
    scalar L2 table only cols 0:2 hold data (lhsT never reads the rest).

Math identical to reference:
  per layer: agg = dinv (.) ((A+I) (dinv (.) t)), dinv = rsqrt(indeg+1).
  L1 t = x (scalar), L2 t = h1 (rank 2 -> 2 scalar channels), L3 t = h2@W3.
"""

import os
import sys
from contextlib import ExitStack

import numpy as np

for _p in ("/opt/trn_rl_repo",):
    if _p not in sys.path and os.path.isdir(_p):
        sys.path.insert(0, _p)

import concourse.bass as bass
import concourse.mybir as mybir
import concourse.tile as tile
from concourse import bacc
from concourse import bass_utils
from concourse.masks import make_identity
from concourse.tile_rust import add_dep_helper

F32 = mybir.dt.float32
I16 = mybir.dt.int16
AF = mybir.ActivationFunctionType
OP = mybir.AluOpType

N_NODES = 50000
N_EDGES = 600000
N_GRAPHS = 256
C = 8
NW = 49
NWS = NW * 128          # 6272 nodes per core
PAD_N = C * NWS         # 50176
HALF = 32768
GP = 256
GSZ = 4                 # windows per gather group


def wrap16(vals):
    return np.ascontiguousarray(vals.reshape(-1, 16).T.astype(np.int16))


def rep8(block):
    return np.tile(block, (8, 1))


class Plan:
    """Per-core edge packing metadata (topology-derived, shared across cores
    as parallel lists)."""

    def __init__(self, edge_index):
        src = np.asarray(edge_index[0], dtype=np.int64)
        dst = np.asarray(edge_index[1], dtype=np.int64)
        self.deg = np.bincount(dst, minlength=N_NODES).astype(np.float32)

        core_of = dst // NWS
        order0 = np.argsort(core_of, kind="stable")
        src, dst, core_sorted = src[order0], dst[order0], core_of[order0]

        self.cores = []
        for c in range(C):
            base = c * NWS
            lo = np.searchsorted(core_sorted, c, side="left")
            hi = np.searchsorted(core_sorted, c, side="right")
            c_src, c_dst = src[lo:hi], dst[lo:hi]
            w_of = (c_dst - base) >> 7
            is_a = c_src < HALF
            key = w_of * 2 + (~is_a)
            order = np.argsort(key, kind="stable")
            c_src, c_dst, w_of, is_a, key = (
                c_src[order], c_dst[order], w_of[order], is_a[order],
                key[order])
            kstart = np.searchsorted(key, np.arange(2 * NW), side="left")
            kend = np.searchsorted(key, np.arange(2 * NW), side="right")
            cnt = kend - kstart
            cntA, cntB = cnt[0::2], cnt[1::2]
            ntA = np.ceil(cntA / 128).astype(int)
            ntB = np.ceil(cntB / 128).astype(int)
            nt = ntA + ntB
            rank = np.arange(len(key)) - kstart[key]  # rank within (w, half)

            tbase = np.concatenate([[0], np.cumsum(nt)]).astype(int)
            T_total = int(tbase[-1])

            # groups
            groups = []
            offA = [0]
            offB = [0]
            for ws in range(0, NW, GSZ):
                gn = min(GSZ, NW - ws)
                wl = np.arange(ws, ws + gn)
                gA = int(ntA[wl].sum())
                gB = int(ntB[wl].sum())
                cA0 = np.concatenate([[0], np.cumsum(ntA[wl])]).astype(int)
                cB0 = np.concatenate([[0], np.cumsum(ntB[wl])]).astype(int)
                groups.append(dict(ws=ws, gn=gn, gA=gA, gB=gB,
                                   cA0=cA0, cB0=cB0,
                                   ntA=ntA[wl].copy(), ntB=ntB[wl].copy(),
                                   colA=offA[-1], colB=offB[-1]))
                offA.append(offA[-1] + gA * 128 // 16)
                offB.append(offB[-1] + gB * 128 // 16)
            TOT_A, TOT_B = offA[-1], offB[-1]

            # per-edge global tile index + lane
            gt = np.where(
                is_a,
                tbase[w_of] + rank // 128,
                tbase[w_of] + ntA[w_of] + rank // 128)
            lane = rank % 128

            # per-edge position within the group idx sequences
            gi_of = w_of // GSZ
            grpA_off = np.zeros(NW, int)
            grpB_off = np.zeros(NW, int)
            for g in groups:
                ws, gn = g["ws"], g["gn"]
                grpA_off[ws:ws + gn] = g["cA0"][:gn] * 128
                grpB_off[ws:ws + gn] = g["cB0"][:gn] * 128
            colA_base = np.array([g["colA"] * 16 for g in groups])
            colB_base = np.array([g["colB"] * 16 for g in groups])
            posA = colA_base[gi_of] + grpA_off[w_of] + rank
            posB = colB_base[gi_of] + grpB_off[w_of] + rank

            seqA = np.zeros(TOT_A * 16, np.int64)
            seqB = np.zeros(TOT_B * 16, np.int64)
            seqA[posA[is_a]] = c_src[is_a]
            seqB[posB[~is_a]] = c_src[~is_a] - HALF

            self.cores.append(dict(
                base=base, src=c_src, dst=c_dst, w_of=w_of, is_a=is_a,
                rank=rank, ntA=ntA, ntB=ntB, nt=nt, tbase=tbase,
                T_total=T_total, groups=groups, TOT_A=TOT_A, TOT_B=TOT_B,
                gt=gt, lane=lane, seqA=seqA, seqB=seqB))

        self.maxT = max(cc["T_total"] for cc in self.cores)
        self.maxNT = max(int(cc["nt"].max()) for cc in self.cores)
        self.maxTOT_A = max(cc["TOT_A"] for cc in self.cores)
        self.maxTOT_B = max(cc["TOT_B"] for cc in self.cores)
        self.max_gC = max(g["gA"] + g["gB"]
                          for cc in self.cores for g in cc["groups"])
        # the device program is built once from core 0's plan shape; all
        # cores must share the same unrolled structure -> pad counts to the
        # max across cores.  Instead of that complexity: build per-core
        # programs?  SPMD needs ONE program.  So we equalize the plan:
        self._equalize()

    def _equalize(self):
        """Pad every core's per-window tile counts up to the max across
        cores so a single SPMD program fits all cores."""
        ntA_max = np.max([cc["ntA"] for cc in self.cores], axis=0)
        ntB_max = np.max([cc["ntB"] for cc in self.cores], axis=0)
        nt = ntA_max + ntB_max
        tbase = np.concatenate([[0], np.cumsum(nt)]).astype(int)
        T_total = int(tbase[-1])
        groups = []
        offA = [0]
        offB = [0]
        for ws in range(0, NW, GSZ):
            gn = min(GSZ, NW - ws)
            wl = np.arange(ws, ws + gn)
            gA = int(ntA_max[wl].sum())
            gB = int(ntB_max[wl].sum())
            cA0 = np.concatenate([[0], np.cumsum(ntA_max[wl])]).astype(int)
            cB0 = np.concatenate([[0], np.cumsum(ntB_max[wl])]).astype(int)
            groups.append(dict(ws=ws, gn=gn, gA=gA, gB=gB, cA0=cA0, cB0=cB0,
                               ntA=ntA_max[wl].copy(), ntB=ntB_max[wl].copy(),
                               colA=offA[-1], colB=offB[-1]))
            offA.append(offA[-1] + gA * 128 // 16)
            offB.append(offB[-1] + gB * 128 // 16)
        TOT_A, TOT_B = offA[-1], offB[-1]

        grpA_off = np.zeros(NW, int)
        grpB_off = np.zeros(NW, int)
        colA_base = np.zeros(NW, int)
        colB_base = np.zeros(NW, int)
        for g in groups:
            ws, gn = g["ws"], g["gn"]
            grpA_off[ws:ws + gn] = g["cA0"][:gn] * 128
            grpB_off[ws:ws + gn] = g["cB0"][:gn] * 128
            colA_base[ws:ws + gn] = g["colA"] * 16
            colB_base[ws:ws + gn] = g["colB"] * 16

        for cc in self.cores:
            w_of, is_a, rank = cc["w_of"], cc["is_a"], cc["rank"]
            gt = np.where(is_a,
                          tbase[w_of] + rank // 128,
                          tbase[w_of] + ntA_max[w_of] + rank // 128)
            posA = colA_base[w_of] + grpA_off[w_of] + rank
            posB = colB_base[w_of] + grpB_off[w_of] + rank
            seqA = np.zeros(TOT_A * 16, np.int64)
            seqB = np.zeros(TOT_B * 16, np.int64)
            seqA[posA[is_a]] = cc["src"][is_a]
            seqB[posB[~is_a]] = cc["src"][~is_a] - HALF
            cc.update(gt=gt, seqA=seqA, seqB=seqB)

        self.ntA = ntA_max
        self.ntB = ntB_max
        self.nt = nt
        self.tbase = tbase
        self.T_total = T_total
        self.groups = groups
        self.TOT_A = TOT_A
        self.TOT_B = TOT_B
        self.max_gC = max(g["gA"] + g["gB"] for g in groups)

    def key(self):
        return (self.T_total, self.TOT_A, self.TOT_B,
                tuple(self.nt.tolist()))


def prep_inputs(plan, x, W1, b1, W2, b2, W3, b3, lin1_w, lin1_b, lin2_w,
                lin2_b, edge_index, batch):
    assert not np.any(np.asarray(b1)), "v2 exploits b1 == 0 (rank-2 h1)"
    assert not np.any(np.asarray(b2)), "t3local exploits b2 == 0"
    x = np.asarray(x, np.float32).reshape(-1)
    batch = np.asarray(batch, dtype=np.int64)
    deg = plan.deg

    x_ext = np.zeros(PAD_N, np.float32); x_ext[:N_NODES] = x
    deg_ext = np.zeros(PAD_N, np.float32); deg_ext[:N_NODES] = deg
    batch_ext = np.full(PAD_N, -1.0, np.float32)
    batch_ext[:N_NODES] = batch.astype(np.float32)

    iota = np.broadcast_to(np.arange(GP, dtype=np.float32),
                           (128, GP)).copy()
    cnts = np.bincount(batch, minlength=GP).astype(np.float32)
    cnts2 = np.ascontiguousarray(cnts.reshape(2, 128).T)

    in_maps = []
    for c, cc in enumerate(plan.cores):
        T = plan.T_total
        drel = np.full((128, T), -1.0, np.float32)
        ysx = np.zeros((128, T), np.float32)
        ysd = np.zeros((128, T), np.float32)
        gt, lane = cc["gt"], cc["lane"]
        drel[lane, gt] = (cc["dst"] - cc["base"] - (cc["w_of"] << 7)
                          ).astype(np.float32)
        ysx[lane, gt] = x[cc["src"]]
        ysd[lane, gt] = deg[cc["src"]]

        ixa = rep8(wrap16(cc["seqA"]))
        ixb = rep8(wrap16(cc["seqB"]))

        base = cc["base"]
        sl = slice(base, base + NWS)
        nd = lambda a: np.ascontiguousarray(a[sl].reshape(NW, 128).T)
        degT = np.broadcast_to(deg_ext[sl][None, :], (128, NWS)).copy()

        in_maps.append({
            "ixa": ixa, "ixb": ixb,
            "drel": drel, "ysx": ysx, "ysd": ysd,
            "nd_batch": nd(batch_ext),
            "nd_x": nd(x_ext), "nd_deg": nd(deg_ext),
            "degT": degT,
            "iota": iota, "cnts": cnts2,
            "w1": np.asarray(W1, np.float32).reshape(1, 64),
            "w2": np.asarray(W2, np.float32).reshape(64, 128),
            "w3": np.asarray(W3, np.float32).reshape(128, 64),
            "b2": np.asarray(b2, np.float32).reshape(128, 1),
            "b3": np.asarray(b3, np.float32).reshape(64, 1),
            "l1w": np.asarray(lin1_w, np.float32).reshape(64, 32),
            "l1b": np.asarray(lin1_b, np.float32).reshape(32, 1),
            "l2w": np.asarray(lin2_w, np.float32).reshape(32, 1),
            "l2b": np.full((128, 1),
                           np.float32(np.asarray(lin2_b).reshape(())),
                           np.float32),
        })
    return in_maps


# ----------------------------------------------------------------------------
# Device program
# ----------------------------------------------------------------------------

def build_program(plan, reps=1, no_coll=False, no_gather=False,
                  no_mm=False, no_oh=False, t3local=False):
    rg = [list(range(C))]
    T = plan.T_total
    TOT_A, TOT_B = plan.TOT_A, plan.TOT_B
    MAXNT = int(plan.nt.max())
    MAXGC = plan.max_gC

    nc = bacc.Bacc("TRN2", target_bir_lowering=False, debug=False,
                   num_devices=C, num_swdge_queues=4)

    din = {}
    def inp(name, shape, dt=F32):
        din[name] = nc.dram_tensor(name, list(shape), dt,
                                   kind="ExternalInput")
        return din[name]

    inp("ixa", (128, TOT_A), I16)
    inp("ixb", (128, TOT_B), I16)
    inp("drel", (128, T)); inp("ysx", (128, T)); inp("ysd", (128, T))
    inp("nd_batch", (128, NW))
    inp("nd_x", (128, NW)); inp("nd_deg", (128, NW))
    inp("degT", (128, NWS))
    inp("iota", (128, GP)); inp("cnts", (128, 2))
    inp("w1", (1, 64)); inp("w2", (64, 128)); inp("w3", (128, 64))
    inp("b2", (128, 1)); inp("b3", (64, 1))
    inp("l1w", (64, 32)); inp("l1b", (32, 1)); inp("l2w", (32, 1))
    inp("l2b", (128, 1))

    out_d = nc.dram_tensor("out", [GP, 1], F32, kind="ExternalOutput")

    t2c_sl = nc.dram_tensor("t2c_slice", [NWS, 2], F32, kind="Internal")
    t2c_full = nc.dram_tensor("t2c_full", [PAD_N, 2], F32, kind="Internal",
                              addr_space="Shared")
    t2 = nc.dram_tensor("t2", [PAD_N, 64], F32, kind="Internal")
    y3_sl = nc.dram_tensor("y3_slice", [NWS, 64], F32, kind="Internal")
    y3_full = nc.dram_tensor("y3_full", [PAD_N, 64], F32, kind="Internal",
                             addr_space="Shared")
    t3c_sl = nc.dram_tensor("t3c_slice", [2, NWS], F32, kind="Internal")
    t3c_full = nc.dram_tensor("t3c_full", [2 * C, NWS], F32, kind="Internal",
                              addr_space="Shared")
    t3 = nc.dram_tensor("t3", [PAD_N, 64], F32, kind="Internal")
    pool_in = nc.dram_tensor("pool_in", [GP, 64], F32, kind="Internal")
    pool_out = nc.dram_tensor("pool_out", [GP, 64], F32, kind="Internal",
                              addr_space="Shared")

    with tile.TileContext(nc) as tc, ExitStack() as ctx:
        P = ctx.enter_context
        setup = P(tc.tile_pool(name="setup", bufs=1))
        oh_pool = P(tc.tile_pool(name="oh", bufs=2))
        gb_pool = P(tc.tile_pool(name="gb", bufs=2))
        fn_pool = P(tc.tile_pool(name="fn", bufs=2))
        psS = P(tc.tile_pool(name="psS", bufs=2, space="PSUM"))
        psZ = P(tc.tile_pool(name="psZ", bufs=2, space="PSUM"))
        psT = P(tc.tile_pool(name="psT", bufs=2, space="PSUM"))
        psHold = P(tc.tile_pool(name="psHold", bufs=1, space="PSUM"))
        ev1 = P(tc.tile_pool(name="ev1", bufs=3))
        ev2 = P(tc.tile_pool(name="ev2", bufs=3))
        ev3 = P(tc.tile_pool(name="ev3", bufs=3))
        stg = P(tc.tile_pool(name="stg", bufs=1))
        expp = P(tc.tile_pool(name="expp", bufs=2))

        def load(name, shape, dt=F32):
            t = setup.tile(list(shape), dt, tag=name)
            nc.sync.dma_start(out=t[:], in_=din[name].ap()[:])
            return t

        ixa = load("ixa", (128, TOT_A), I16)
        ixb = load("ixb", (128, TOT_B), I16)
        drel = load("drel", (128, T))
        ysx = load("ysx", (128, T))
        ysd = load("ysd", (128, T))
        nd_batch = load("nd_batch", (128, NW))
        nd_x = load("nd_x", (128, NW))
        nd_deg = load("nd_deg", (128, NW))
        degT = load("degT", (128, NWS))
        iota = load("iota", (128, GP))
        cnts = load("cnts", (128, 2))
        w1 = load("w1", (1, 64)); w2 = load("w2", (64, 128))
        w3 = load("w3", (128, 64))
        b2 = load("b2", (128, 1)); b3 = load("b3", (64, 1))
        l1w = load("l1w", (64, 32)); l1b = load("l1b", (32, 1))
        l2w = load("l2w", (32, 1)); l2b = load("l2b", (128, 1))

        ident = setup.tile([128, 128], F32, tag="ident")
        make_identity(nc, ident[:])
        fence_ix = setup.tile([128, 8], I16, tag="fence_ix")
        nc.vector.memset(fence_ix[:], 0)

        # dinvT = rsqrt(degT + 1) computed in place (free dim layout)
        dinvT = degT
        nc.scalar.activation(out=dinvT[:], in_=degT[:], func=AF.Sqrt,
                             bias=1.0, scale=1.0)
        nc.vector.reciprocal(out=dinvT[:], in_=dinvT[:])

        # per-slot src normalization: ys = x[src] * rsqrt(deg[src]+1)
        nc.scalar.activation(out=ysd[:], in_=ysd[:], func=AF.Sqrt,
                             bias=1.0, scale=1.0)
        nc.vector.reciprocal(out=ysd[:], in_=ysd[:])
        ys = ysx
        nc.vector.tensor_tensor(out=ys[:], in0=ysd[:], in1=ysx[:],
                                op=OP.mult)

        # own-node x*dinv, node-partition layout (L1 self-loop lhsT)
        dinv_nm = setup.tile([128, NW], F32, tag="dinv_nm")
        nc.scalar.activation(out=dinv_nm[:], in_=nd_deg[:], func=AF.Sqrt,
                             bias=1.0, scale=1.0)
        nc.vector.reciprocal(out=dinv_nm[:], in_=dinv_nm[:])
        xd_nm = setup.tile([128, NW], F32, tag="xd_nm")
        nc.vector.tensor_tensor(out=xd_nm[:], in0=nd_x[:], in1=dinv_nm[:],
                                op=OP.mult)

        # UVT = [u; v] = [relu(w1); relu(-w1)] @ W2  -> [2, 128]
        w1p = setup.tile([1, 64], F32, tag="w1p")
        w1m = setup.tile([1, 64], F32, tag="w1m")
        nc.scalar.activation(out=w1p[:], in_=w1[:], func=AF.Relu)
        nc.scalar.activation(out=w1m[:], in_=w1[:], func=AF.Relu,
                             scale=-1.0)
        w1pmT = setup.tile([64, 2], F32, tag="w1pmT")
        for i, src_t in enumerate((w1p, w1m)):
            psx = psT.tile([128, 128], F32, space="PSUM", tag="psN")
            nc.tensor.transpose(out=psx[:64, :1], in_=src_t[:],
                                identity=ident[:1, :1])
            nc.scalar.activation(out=w1pmT[:, i:i + 1], in_=psx[:64, :1],
                                 func=AF.Copy)
        psUV = psZ.tile([128, 256], F32, space="PSUM", tag="psz")
        nc.tensor.matmul(out=psUV[:2, :128], lhsT=w1pmT[:], rhs=w2[:],
                         start=True, stop=True)
        UVT = setup.tile([2, 128], F32, tag="UVT")
        nc.scalar.activation(out=UVT[:], in_=psUV[:2, :128], func=AF.Copy)

        staging = stg.tile([128, NW * 64], F32, tag="staging")
        staging2 = stg.tile([128, NW * 2], F32, tag="staging2")
        if t3local:
            staging2c = stg.tile([2, NWS], F32, tag="staging2c")
        else:
            staging2c = None

        def wsl(w):
            return slice(w * 128, (w + 1) * 128)

        def onehot_win(w):
            ntw = int(plan.nt[w])
            t0 = int(plan.tbase[w])
            oh = oh_pool.tile([128, MAXNT * 128], F32, tag="oh")
            if no_oh or ntw == 0:
                nc.vector.memset(oh[:], 0.0)
                return oh
            dr3 = drel[:, t0:t0 + ntw][:, :, None].to_broadcast(
                [128, ntw, 128])
            io3 = iota[:, None, :128].to_broadcast([128, ntw, 128])
            nc.vector.tensor_tensor(
                out=oh[:, :ntw * 128].rearrange("p (t j) -> p t j", j=128),
                in0=dr3, in1=io3, op=OP.is_equal)
            return oh

        def gather_group(g, table):
            gb = gb_pool.tile([128, MAXGC * 64], F32, tag="gb")
            if no_gather:
                nc.vector.memset(gb[:], 0.0)
                return gb, None
            # split each half across two SWDGE queues (4 total)
            qcalls = {0: [], 1: [], 2: [], 3: []}

            def issue(base_chunk, nchunks, colbase, table_ap, queues):
                if nchunks == 0:
                    return
                h1 = (nchunks // 2) if nchunks > 1 else nchunks
                parts = [(0, h1), (h1, nchunks - h1)] if nchunks > 1 \
                    else [(0, nchunks)]
                for (c0, nc_), qn in zip(parts, queues):
                    if nc_ == 0:
                        continue
                    n = nc_ * 128
                    qcalls[qn].append(nc.gpsimd.dma_gather(
                        out_ap=gb[:, (base_chunk + c0) * 64:
                                  (base_chunk + c0 + nc_) * 64].rearrange(
                            "p (t f) -> p t f", f=64),
                        in_ap=table_ap,
                        idxs_ap=ixa[:, 0:1] if False else
                        _ix[:, colbase + c0 * 8:colbase + (c0 + nc_) * 8],
                        num_idxs=n, num_idxs_reg=n, elem_size=64,
                        single_packet=False, queue_num=qn))

            _ix = ixa
            issue(0, g["gA"], g["colA"], table.ap()[:HALF, :], (0, 2))
            _ix = ixb
            issue(g["gA"], g["gB"], g["colB"], table.ap()[HALF:, :], (1, 3))

            fences = []
            for qn in range(4):
                if not qcalls[qn]:
                    continue
                fence_t = fn_pool.tile([128, 64], F32, tag=f"fence{qn}")
                f = nc.gpsimd.dma_gather(
                    out_ap=fence_t[:].rearrange("p (t f) -> p t f", f=64),
                    in_ap=table.ap()[:HALF, :],
                    idxs_ap=fence_ix[:],
                    num_idxs=128, num_idxs_reg=128, elem_size=64,
                    single_packet=True, queue_num=qn)
                for call in qcalls[qn]:
                    add_dep_helper(f.ins, call.ins, True, f"f{qn}>call")
                fences.append(f)
            fence = fences[0]
            for f in fences[1:]:
                add_dep_helper(fence.ins, f.ins, True, "fence>f")
            return gb, fence

        def gb_col(g, wl, t):
            """gb chunk index for window-in-group wl, window tile t."""
            ntAw = int(g["ntA"][wl])
            if t < ntAw:
                return int(g["cA0"][wl]) + t
            return g["gA"] + int(g["cB0"][wl]) + (t - ntAw)

        def scatter_win(oh, lhs_cols, fence, F, self_lhs=None):
            """lhs_cols: (buf, col) per gather tile; self_lhs: (buf, col)
            whose rhs is the identity (adds own-node values)."""
            ps = psS.tile([64, 128], F32, space="PSUM", tag="psS")
            if no_mm:
                nc.vector.memset(ps[:], 0.0)
                return ps
            n = len(lhs_cols) + (1 if self_lhs is not None else 0)
            for t, (buf, col) in enumerate(lhs_cols):
                mm = nc.tensor.matmul(
                    out=ps[:F, :], lhsT=buf[:, col:col + F],
                    rhs=oh[:, t * 128:(t + 1) * 128],
                    start=(t == 0), stop=(t == n - 1))
                if fence is not None:
                    add_dep_helper(mm.ins, fence.ins, True, "mm>fence")
            if self_lhs is not None:
                buf, col = self_lhs
                nc.tensor.matmul(
                    out=ps[:F, :], lhsT=buf[:, col:col + F],
                    rhs=ident[:128, :128], start=(len(lhs_cols) == 0),
                    stop=True)
            return ps

        for _rep in range(reps):
            # ---- Layer 1 (no gather) ---------------------------------------
            for w in range(NW):
                oh = onehot_win(w)
                cols = [(ys, int(plan.tbase[w]) + t)
                        for t in range(int(plan.nt[w]))]
                ps1 = scatter_win(oh, cols, None, 1, self_lhs=(xd_nm, w))
                a = ev1.tile([1, 128], F32, tag="a")
                nc.vector.tensor_tensor(out=a[:], in0=ps1[:1, :],
                                        in1=dinvT[:1, wsl(w)], op=OP.mult)
                sp = ev2.tile([1, 128], F32, tag="sp")
                nc.scalar.activation(out=sp[:], in_=a[:], func=AF.Relu)
                sm = ev2.tile([1, 128], F32, tag="sm")
                nc.scalar.activation(out=sm[:], in_=a[:], func=AF.Relu,
                                     scale=-1.0)
                spd = ev3.tile([1, 128], F32, tag="spd")
                nc.vector.tensor_tensor(out=spd[:], in0=sp[:],
                                        in1=dinvT[:1, wsl(w)], op=OP.mult)
                smd = ev3.tile([1, 128], F32, tag="smd")
                nc.vector.tensor_tensor(out=smd[:], in0=sm[:],
                                        in1=dinvT[:1, wsl(w)], op=OP.mult)
                psN = psT.tile([128, 128], F32, space="PSUM", tag="psN")
                nc.tensor.transpose(out=psN[:, 0:1], in_=spd[:],
                                    identity=ident[:1, :1])
                nc.tensor.transpose(out=psN[:, 1:2], in_=smd[:],
                                    identity=ident[:1, :1])
                nc.scalar.activation(out=staging2[:, w * 2:(w + 1) * 2],
                                     in_=psN[:, :2], func=AF.Copy)

            nc.sync.dma_start(
                out=t2c_sl.ap()[:].rearrange("(w p) f -> p w f", p=128),
                in_=staging2[:].rearrange("p (w f) -> p w f", f=2))
            if no_coll:
                nc.gpsimd.dma_start(out=t2c_full.ap()[:NWS, :],
                                    in_=t2c_sl.ap()[:])
            else:
                nc.gpsimd.collective_compute(
                    "AllGather", OP.bypass, replica_groups=rg,
                    ins=[t2c_sl.ap()[:]], outs=[t2c_full.ap()[:]])
            # expand compact [N,2] into full 256B rows (s+,s- repeated) via
            # SBUF bounce -- avoids a 50k-descriptor strided DRAM write
            t2sb = stg.tile([128, 2 * PAD_N // 128], F32, tag="t2sb")
            nc.sync.dma_start(
                out=t2sb[:].rearrange("p (w f) -> p w f", f=2),
                in_=t2c_full.ap()[:].rearrange("(w p) f -> p w f", p=128))
            for ch in range(8):
                exp = expp.tile([128, NW * 64], F32, tag="exp")
                seg = t2sb[:, ch * 2 * NW:(ch + 1) * 2 * NW].rearrange(
                    "p (w f) -> p w f", f=2)
                nc.vector.tensor_copy(
                    out=exp[:].rearrange("p (w d f) -> p w d f", d=32, f=2),
                    in_=seg[:, :, None, :].to_broadcast([128, NW, 32, 2]))
                nc.sync.dma_start(
                    out=t2.ap()[ch * NWS:(ch + 1) * NWS, :].rearrange(
                        "(w p) f -> p w f", p=128),
                    in_=exp[:].rearrange("p (w f) -> p w f", f=64))

            # ---- Layer 2 ---------------------------------------------------
            for g in plan.groups:
                gb, fence = gather_group(g, t2)
                for wl in range(g["gn"]):
                    w = g["ws"] + wl
                    oh = onehot_win(w)
                    cols = [(gb, gb_col(g, wl, t) * 64)
                            for t in range(int(plan.nt[w]))]
                    ps2 = scatter_win(oh, cols, fence, 2,
                                      self_lhs=(staging2, w * 2))
                    pq = ev1.tile([2, 128], F32, tag="pq")
                    nc.vector.tensor_tensor(out=pq[:], in0=ps2[:2, :],
                                            in1=dinvT[:2, wsl(w)],
                                            op=OP.mult)
                    if t3local:
                        # p~,q~ = dinv * (p,q); table row built locally later
                        nc.vector.tensor_tensor(out=staging2c[:, wsl(w)],
                                                in0=pq[:],
                                                in1=dinvT[:2, wsl(w)],
                                                op=OP.mult)
                    psh = psZ.tile([128, 256], F32, space="PSUM", tag="psz")
                    nc.tensor.matmul(out=psh[:, :128], lhsT=UVT[:], rhs=pq[:],
                                     start=True, stop=True)
                    h2 = ev2.tile([128, 128], F32, tag="h2")
                    nc.scalar.activation(out=h2[:], in_=psh[:, :128],
                                         func=AF.Relu, bias=b2[:])
                    pst3 = psZ.tile([128, 256], F32, space="PSUM", tag="psz")
                    nc.tensor.matmul(out=pst3[:64, :128], lhsT=w3[:],
                                     rhs=h2[:], start=True, stop=True)
                    g3 = ev3.tile([64, 128], F32, tag="g3")
                    nc.vector.tensor_tensor(out=g3[:],
                                            in0=pst3[:64, :128],
                                            in1=dinvT[:64, wsl(w)],
                                            op=OP.mult)
                    psN = psT.tile([128, 128], F32, space="PSUM", tag="psN")
                    nc.tensor.transpose(out=psN[:, :64], in_=g3[:],
                                        identity=ident[:64, :64])
                    nc.scalar.activation(out=staging[:, w * 64:(w + 1) * 64],
                                         in_=psN[:, :64], func=AF.Copy)

            if t3local:
                nc.sync.dma_start(out=t3c_sl.ap()[:], in_=staging2c[:])
                if no_coll:
                    nc.gpsimd.dma_start(out=t3c_full.ap()[:2, :],
                                        in_=t3c_sl.ap()[:])
                else:
                    nc.gpsimd.collective_compute(
                        "AllGather", OP.bypass, replica_groups=rg,
                        ins=[t3c_sl.ap()[:]], outs=[t3c_full.ap()[:]])
                # local T3 build: row n = relu(p~ u + q~ v) @ W3  (b2 == 0)
                for c8 in range(C):
                    pqsb = stg.tile([2, NWS], F32, tag="pqsb")
                    nc.sync.dma_start(out=pqsb[:],
                                      in_=t3c_full.ap()[2 * c8:2 * c8 + 2, :])
                    tstag = expp.tile([128, NW * 64], F32, tag="exp")
                    for wc in range(NW):
                        psh = psZ.tile([128, 256], F32, space="PSUM",
                                       tag="psz")
                        nc.tensor.matmul(out=psh[:, :128], lhsT=UVT[:],
                                         rhs=pqsb[:, wc * 128:(wc + 1) * 128],
                                         start=True, stop=True)
                        h2b = ev2.tile([128, 128], F32, tag="h2")
                        nc.scalar.activation(out=h2b[:], in_=psh[:, :128],
                                             func=AF.Relu)
                        pst = psZ.tile([128, 256], F32, space="PSUM",
                                       tag="psz")
                        nc.tensor.matmul(out=pst[:64, :128], lhsT=w3[:],
                                         rhs=h2b[:], start=True, stop=True)
                        t3v = ev3.tile([64, 128], F32, tag="g3")
                        nc.scalar.activation(out=t3v[:], in_=pst[:64, :128],
                                             func=AF.Copy)
                        psN = psT.tile([128, 128], F32, space="PSUM",
                                       tag="psN")
                        nc.tensor.transpose(out=psN[:, :64], in_=t3v[:],
                                            identity=ident[:64, :64])
                        nc.scalar.activation(
                            out=tstag[:, wc * 64:(wc + 1) * 64],
                            in_=psN[:, :64], func=AF.Copy)
                    nc.sync.dma_start(
                        out=t3.ap()[c8 * NWS:(c8 + 1) * NWS, :].rearrange(
                            "(w p) f -> p w f", p=128),
                        in_=tstag[:].rearrange("p (w f) -> p w f", f=64))
            else:
                nc.sync.dma_start(
                    out=y3_sl.ap()[:].rearrange("(w p) f -> p w f", p=128),
                    in_=staging[:].rearrange("p (w f) -> p w f", f=64))
                if no_coll:
                    nc.gpsimd.dma_start(out=y3_full.ap()[:NWS, :],
                                        in_=y3_sl.ap()[:])
                else:
                    nc.gpsimd.collective_compute(
                        "AllGather", OP.bypass, replica_groups=rg,
                        ins=[y3_sl.ap()[:]], outs=[y3_full.ap()[:]])

            # ---- Layer 3 + pooling ----------------------------------------
            pooled_a = psHold.tile([128, 64], F32, space="PSUM", tag="pool_a")
            pooled_b = psHold.tile([128, 64], F32, space="PSUM", tag="pool_b")
            for g in plan.groups:
                gb, fence = gather_group(g, t3 if t3local else y3_full)
                for wl in range(g["gn"]):
                    w = g["ws"] + wl
                    oh = onehot_win(w)
                    cols = [(gb, gb_col(g, wl, t) * 64)
                            for t in range(int(plan.nt[w]))]
                    ps3 = scatter_win(oh, cols, fence, 64,
                                      self_lhs=(staging, w * 64))
                    agg = ev1.tile([64, 128], F32, tag="agg")
                    nc.vector.tensor_tensor(out=agg[:], in0=ps3[:64, :],
                                            in1=dinvT[:64, wsl(w)],
                                            op=OP.mult)
                    h3 = ev2.tile([64, 128], F32, tag="h3")
                    nc.scalar.activation(out=h3[:], in_=agg[:], func=AF.Relu,
                                         bias=b3[:])
                    psN = psT.tile([128, 128], F32, space="PSUM", tag="psN")
                    nc.tensor.transpose(out=psN[:, :64], in_=h3[:],
                                        identity=ident[:64, :64])
                    h3nm = ev3.tile([128, 64], F32, tag="h3nm")
                    nc.scalar.activation(out=h3nm[:], in_=psN[:, :64],
                                         func=AF.Copy)
                    ohp = oh_pool.tile([128, GP], F32, tag="ohp")
                    bc = nd_batch[:, w:w + 1].to_broadcast([128, GP])
                    nc.vector.tensor_tensor(out=ohp[:], in0=bc, in1=iota[:],
                                            op=OP.is_equal)
                    for half, ps_pool in ((0, pooled_a), (1, pooled_b)):
                        lhs = ohp[:, half * 128:(half + 1) * 128]
                        nc.tensor.matmul(out=ps_pool[:], lhsT=lhs,
                                         rhs=h3nm[:],
                                         start=(w == 0), stop=(w == NW - 1))

            # ---- finale ----------------------------------------------------
            pa = setup.tile([128, 64], F32, tag="pa")
            pb = setup.tile([128, 64], F32, tag="pb")
            nc.scalar.activation(out=pa[:], in_=pooled_a[:], func=AF.Copy)
            nc.scalar.activation(out=pb[:], in_=pooled_b[:], func=AF.Copy)
            nc.sync.dma_start(out=pool_in.ap()[0:128, :], in_=pa[:])
            nc.sync.dma_start(out=pool_in.ap()[128:256, :], in_=pb[:])
            if no_coll:
                nc.gpsimd.dma_start(out=pool_out.ap()[:],
                                    in_=pool_in.ap()[:])
            else:
                nc.gpsimd.collective_compute(
                    "AllReduce", OP.add, replica_groups=rg,
                    ins=[pool_in.ap()[:]], outs=[pool_out.ap()[:]])

        meanT = setup.tile([64, 256], F32, tag="meanT")
        for half in (0, 1):
            pl = setup.tile([128, 64], F32, tag=f"pl{half}")
            nc.sync.dma_start(
                out=pl[:], in_=pool_out.ap()[half * 128:(half + 1) * 128, :])
            cntm = setup.tile([128, 1], F32, tag=f"cntm{half}")
            nc.vector.tensor_scalar_max(out=cntm[:],
                                        in0=cnts[:, half:half + 1],
                                        scalar1=1.0)
            rc = setup.tile([128, 1], F32, tag=f"rc{half}")
            nc.vector.reciprocal(out=rc[:], in_=cntm[:])
            mean = setup.tile([128, 64], F32, tag=f"mean{half}")
            nc.vector.tensor_scalar_mul(out=mean[:], in0=pl[:],
                                        scalar1=rc[:])
            psMT = psT.tile([128, 128], F32, space="PSUM", tag="psN")
            nc.tensor.transpose(out=psMT[:64, :], in_=mean[:],
                                identity=ident[:])
            nc.scalar.activation(out=meanT[:, half * 128:(half + 1) * 128],
                                 in_=psMT[:64, :], func=AF.Copy)

        psZ1 = psZ.tile([128, 256], F32, space="PSUM", tag="psz")
        nc.tensor.matmul(out=psZ1[:32, :256], lhsT=l1w[:], rhs=meanT[:],
                         start=True, stop=True)
        z1 = setup.tile([32, 256], F32, tag="z1")
        nc.scalar.activation(out=z1[:], in_=psZ1[:32, :256], func=AF.Relu,
                             bias=l1b[:])
        for half in (0, 1):
            psO = psT.tile([128, 128], F32, space="PSUM", tag="psN")
            nc.tensor.matmul(out=psO[:, :1],
                             lhsT=z1[:, half * 128:(half + 1) * 128],
                             rhs=l2w[:], start=True, stop=True)
            ob = setup.tile([128, 1], F32, tag=f"ob{half}")
            nc.scalar.activation(out=ob[:], in_=psO[:, :1],
                                 func=AF.Identity, bias=l2b[:])
            nc.sync.dma_start(out=out_d.ap()[half * 128:(half + 1) * 128, :],
                              in_=ob[:])

    nc.compile()
    return nc


# ----------------------------------------------------------------------------
# Runner
# ----------------------------------------------------------------------------

_CACHE = {}


def get_program(plan, reps=1, **kw):
    key = plan.key() + (reps,) + tuple(sorted(kw.items()))
    if key not in _CACHE:
        _CACHE[key] = build_program(plan, reps, **kw)
    return _CACHE[key]


def run(plan, inputs, trace=False):
    in_maps = prep_inputs(plan, **inputs)
    nc = get_program(plan)
    res = bass_utils.run_bass_kernel_spmd(
        nc, in_maps, core_ids=list(range(C)), trace=trace)
    out = res.results[0]["out"][:N_GRAPHS, :].astype(np.float32)
    return out, res


def kernel(**inputs) -> np.ndarray:
    plan = Plan(inputs["edge_index"])
    out, _ = run(plan, inputs)
    return out
